# revision 25
# baseline (speedup 1.0000x reference)
"""Trainium2 Bass kernel for the McSharry-style ECG Euler integrator (v3).

Problem (hardcoded): B=131072 beats, params x[B,15] = interleaved (a,b,theta)
x 5 gaussian waves, v0[B] initial z; 216 Euler steps; per-row min/max rescale.

The (x,y) orbit is batch-independent -> th(t), z0(t) are 216-entry host
tables, and per row  z_{t+1} = c*z_t + u_t  with
    u_t = hz0_t - H * sum_i a_i * g_{s_i,theta_i}(th_t),
    g_{s,th0}(th) = (th-th0) * exp(-s^2 (th-th0)^2),  s = min(1/(sqrt2*b), 1e3).

v3 insight: g has an ANALYTIC Fourier transform, so on a period-12 domain
    g(th) = sum_k 2*A_k(s) * sin(w_k (th - th0)),  w_k = 2 pi k / 12,
    A_k = w_k sqrt(pi)/(2 L s^3) e^{-w_k^2/(4 s^2)}
truncated at K=127 harmonics: numerically exact for s <= ~13.  The host
folds the whole 5-wave sum into per-row sin/cos coefficients, so on
device u comes from TWO f16 128x128 matmuls against fixed sin/cos
tables -- no per-wave work at all.  Rows where some s > 13 ("narrow"
waves, ~24%) are sorted into correction tiles: their narrow waves are
evaluated directly via a bf16 hi/lo-split PE matmul (x = s*(th-th0),
adth) + one ACT Derivative_Erf + a short f16 DVE chain, added onto the
spectral u.  Rows with 3+ narrow waves (~0.2%) go to one overflow tile
that runs the correction path three times.

Engine split: PE 2 f16 matmuls/tile (+1 bf16 for corr); ACT DErf (corr)
+ per-tile rescale; DVE z-scan, f16 min/max tensor_reduce per 8-tile
group, corr q-chain.  GPSIMD stays idle -- measured: its SBUF traffic
contends with DVE and slows every DVE op ~3x.
Output f16, upcast on host.  Sharding: data-parallel over 8 cores with
a host-side row permutation (narrow rows dealt evenly), inverted after.
"""

import math
import numpy as np

# ---------------------------------------------------------------- constants
B_FULL = 131072
N_CORES = 8
B_SHARD = B_FULL // N_CORES      # 16384
NT = 216                         # time steps
NW = 5                           # gaussian waves
P = 128                          # partitions
NTILES_FULL = B_SHARD // P       # 128 row-tiles per core
KF = 127                         # Fourier harmonics
LPER = 12.0                      # Fourier period
S_STAR = 13.0                    # narrow-wave threshold
KX = 8                           # x coeff rows per corr slot
KA = 5                           # adth coeff rows per corr slot
NSLOT = 2                        # narrow slots per corr tile row
KC = NSLOT * (KX + KA)           # 26 corr stationary rows

H = 1.0 / 216.0
A_Z0 = 0.005
F2 = 0.25
OMEGA = 2.0 * math.pi
X0 = -0.417750770388669
Y0 = -0.9085616622823985
MIN_VAL = -0.01563
MAX_VAL = 0.042557
SQRT2 = math.sqrt(2.0)
SG_MAX = 1.0e3
SPI2 = math.sqrt(math.pi) / 2.0


def _tile_split(ntiles):
    """(W, C, F) tile counts per core."""
    if ntiles >= 16:
        C = max(2, int(math.ceil(ntiles * 0.265)))
        F = 1
    else:
        C = max(1, ntiles // 4)
        F = 1
    W = ntiles - C - F
    assert W >= 1
    return W, C, F


def _tile_layout(ntiles):
    """Per-tile type layout: corr tiles spread evenly, overflow tile last.
    Returns (types list 'W'/'C'/'F', corr_rank dict g->slice index)."""
    W, C, F = _tile_split(ntiles)
    types = ['W'] * ntiles
    types[ntiles - 1] = 'F'
    pos = [int((i + 0.5) * (ntiles - 1) / C) for i in range(C)]
    # resolve collisions while keeping order
    used = {ntiles - 1}
    corr_pos = []
    for p_ in pos:
        while p_ in used:
            p_ += 1
        assert p_ <= ntiles - 2
        used.add(p_)
        corr_pos.append(p_)
    for p_ in corr_pos:
        types[p_] = 'C'
    corr_rank = {g: i for i, g in enumerate(sorted(corr_pos))}
    return types, corr_rank


def _host_tables():
    """Replicate the reference's fp32 (x,y) Euler orbit -> th, z0 tables."""
    h = np.float32(H)
    om = np.float32(OMEGA)
    one = np.float32(1.0)
    x = np.float32(X0)
    y = np.float32(Y0)
    th = np.empty(NT, np.float32)
    for k in range(NT):
        th[k] = np.arctan2(y, x)
        r = np.sqrt(x * x + y * y)
        alpha = one - r
        fx = alpha * x - om * y
        fy = alpha * y + om * x
        x = x + h * fx
        y = y + h * fy
    t = np.arange(NT, dtype=np.float32) / np.float32(216.0)
    z0 = np.float32(A_Z0) * np.sin(np.float32(2.0 * math.pi * F2) * t)
    return th, z0


def _build_program(ntiles=NTILES_FULL):
    import concourse.bacc as bacc
    import concourse.tile as tile
    from concourse import mybir

    f32 = mybir.dt.float32
    f16 = mybir.dt.float16
    bf16 = mybir.dt.bfloat16
    Act = mybir.ActivationFunctionType
    Op = mybir.AluOpType
    X = mybir.AxisListType.X

    rows = ntiles * P
    W, C, F = _tile_split(ntiles)
    NCOR = C + 3 * F                 # corr stationary slices (F = 3 passes)
    GB = 8 if ntiles % 8 == 0 else (4 if ntiles % 4 == 0 else 1)
    assert ntiles % GB == 0

    nc = bacc.Bacc("TRN2", target_bir_lowering=False, debug=False,
                   num_devices=N_CORES)

    SG = 4                           # tiles per batched scan (PSUM group)
    NTP = 256                        # padded per-tile column slot
    assert ntiles % SG == 0 and GB % SG == 0

    lhs1_d = nc.declare_dram_parameter("lhs1", [P, rows], f16, isOutput=False)
    lhs2_d = nc.declare_dram_parameter("lhs2", [P, rows], f16, isOutput=False)
    b1_d = nc.declare_dram_parameter("b1", [P, NTP], f16, isOutput=False)
    b2_d = nc.declare_dram_parameter("b2", [P, NTP], f16, isOutput=False)
    clhs_d = nc.declare_dram_parameter("clhs", [KC, NCOR * P], bf16, isOutput=False)
    cbas_d = nc.declare_dram_parameter("cbas", [KC, 4 * NT], bf16, isOutput=False)
    cb4_d = nc.declare_dram_parameter("cb4", [P, SG * NTP], f32, isOutput=False)
    out_d = nc.declare_dram_parameter("out", [rows, NT], f16, isOutput=True)

    with tile.TileContext(nc) as tc:
        with tc.tile_pool(name="consts", bufs=1) as consts, \
             tc.tile_pool(name="work", bufs=6) as work, \
             tc.tile_pool(name="zp", bufs=3) as zp, \
             tc.tile_pool(name="outp", bufs=3) as outp, \
             tc.tile_pool(name="ups", bufs=2, space="PSUM") as upsp, \
             tc.tile_pool(name="cps", bufs=2, space="PSUM") as cpsp:

            LHS1 = consts.tile([P, rows], f16)
            LHS2 = consts.tile([P, rows], f16)
            NCH = 8 if ntiles % 8 == 0 else 1
            chunk = rows // NCH
            for cc in range(NCH):
                nc.sync.dma_start(out=LHS1[:, cc * chunk:(cc + 1) * chunk],
                                  in_=lhs1_d[:, cc * chunk:(cc + 1) * chunk])
                nc.sync.dma_start(out=LHS2[:, cc * chunk:(cc + 1) * chunk],
                                  in_=lhs2_d[:, cc * chunk:(cc + 1) * chunk])
            B1 = consts.tile([P, NTP], f16)
            nc.sync.dma_start(out=B1, in_=b1_d[:, :])
            B2 = consts.tile([P, NTP], f16)
            nc.sync.dma_start(out=B2, in_=b2_d[:, :])
            CLHS = consts.tile([KC, NCOR * P], bf16)
            nc.sync.dma_start(out=CLHS, in_=clhs_d[:, :])
            CBAS = consts.tile([KC, 4 * NT], bf16)
            nc.sync.dma_start(out=CBAS, in_=cbas_d[:, :])
            CB4 = consts.tile([P, SG * NTP], f32)
            nc.sync.dma_start(out=CB4, in_=cb4_d[:, :])
            MINV = consts.tile([P, GB], f32)
            nc.vector.memset(MINV, MIN_VAL)

            RING = 4
            e2r = [consts.tile([P, 2 * NT], f16, name=f"e2r{k}")
                   for k in range(RING)]
            q2r = [consts.tile([P, 2 * NT], f16, name=f"q2r{k}")
                   for k in range(RING)]

            def corr_pass(idx, g, ups_slot):
                """Add 2 narrow-wave slots (stationary slice idx) into the
                PSUM u slot [P,NT] of tile g, in place."""
                CPS = cpsp.tile([P, 1024], f32, tag="cps")
                cl = CLHS[:, idx * P:(idx + 1) * P]
                nc.tensor.matmul(CPS[:, 0:432], cl, CBAS[:, 0:432],
                                 start=True, stop=True)
                nc.tensor.matmul(CPS[:, 512:944], cl, CBAS[:, 432:864],
                                 start=True, stop=True)
                e2 = e2r[g % RING]
                nc.scalar.activation(e2, CPS[:, 0:432], Act.Derivative_Erf)
                q2 = q2r[g % RING]
                nc.vector.tensor_mul(q2, CPS[:, 512:944], e2)
                qs = work.tile([P, NT], f16, tag="qs")
                nc.vector.tensor_add(qs, q2[:, 0:NT], q2[:, NT:2 * NT])
                nc.vector.tensor_add(ups_slot, ups_slot, qs)

            types, corr_rank = _tile_layout(ntiles)
            n_used_corr = 0
            for gb in range(ntiles // GB):
                ZR = zp.tile([P, GB, NTP], f32, tag="zr")
                for sgi in range(GB // SG):
                    UPSG = upsp.tile([P, SG * NTP], f32, tag="ups")
                    for j in range(SG):
                        g = gb * GB + sgi * SG + j
                        out_sl = UPSG[:, j * NTP:j * NTP + NTP]
                        nc.tensor.matmul(out_sl, LHS1[:, g * P:(g + 1) * P],
                                         B1, start=True, stop=False)
                        nc.tensor.matmul(out_sl, LHS2[:, g * P:(g + 1) * P],
                                         B2, start=False, stop=True)
                        ups_u = UPSG[:, j * NTP:j * NTP + NT]
                        if types[g] == 'C':
                            corr_pass(corr_rank[g], g, ups_u)
                            n_used_corr += 1
                        elif types[g] == 'F':
                            for p_ in range(3):
                                corr_pass(C + p_, g, ups_u)
                                n_used_corr += 1
                    # one segmented scan for SG tiles (v0 injected via the
                    # delta row; c-pattern: 0 at t=0, c in-tile, 1 in pads)
                    nc.vector.tensor_tensor_scan(
                        ZR[:, sgi * SG:(sgi + 1) * SG, :].rearrange(
                            "p j t -> p (j t)"),
                        CB4, UPSG, 0.0, Op.mult, Op.add)

                zmin = work.tile([P, GB], f32, tag="zmin")
                nc.vector.tensor_reduce(zmin, ZR[:, :, 0:NT], axis=X, op=Op.min)
                zmax = work.tile([P, GB], f32, tag="zmax")
                nc.vector.tensor_reduce(zmax, ZR[:, :, 0:NT], axis=X, op=Op.max)
                d4 = work.tile([P, GB], f32, tag="d4")
                nc.vector.tensor_sub(d4, zmax, zmin)
                r4 = work.tile([P, GB], f32, tag="r4")
                nc.vector.reciprocal(r4, d4)
                s4 = work.tile([P, GB], f32, tag="s4")
                nc.vector.tensor_scalar_mul(s4, r4, MAX_VAL)
                t4 = work.tile([P, GB], f32, tag="t4")
                nc.vector.tensor_mul(t4, zmin, s4)
                bo4 = work.tile([P, GB], f32, tag="bo4")
                nc.vector.tensor_sub(bo4, MINV, t4)
                O8 = outp.tile([P, GB, NT], f16, tag="o8")
                for j in range(GB):
                    nc.scalar.activation(O8[:, j, :], ZR[:, j, 0:NT],
                                         Act.Identity,
                                         bias=bo4[:, j:j + 1],
                                         scale=s4[:, j:j + 1])
                nc.sync.dma_start(
                    out=out_d[gb * GB * P:(gb + 1) * GB * P, :].rearrange(
                        "(j p) t -> p j t", p=P),
                    in_=O8)
            assert n_used_corr == NCOR

    nc.compile()
    return nc


_PROG_CACHE = {}


def _get_program(ntiles=NTILES_FULL):
    if ntiles not in _PROG_CACHE:
        _PROG_CACHE[ntiles] = _build_program(ntiles)
    return _PROG_CACHE[ntiles]


def _bf16(x):
    import ml_dtypes
    return np.asarray(x).astype(ml_dtypes.bfloat16)


def _fourier_coeffs(amp, sg, theta, wide):
    """SC, CC [nrows, KF] f32: per-row spectral coefficients of
    sum_i amp_i * (th-theta_i) exp(-sg_i^2 (th-theta_i)^2) over wide waves."""
    n = amp.shape[0]
    SC = np.zeros((n, KF), np.float32)
    CC = np.zeros((n, KF), np.float32)
    wk = (2.0 * np.pi / LPER) * np.arange(1, KF + 1)
    CH = 8192
    for lo in range(0, n, CH):
        hi = min(lo + CH, n)
        a = (amp[lo:hi] * wide[lo:hi]).astype(np.float64)      # [m,5]
        s = sg[lo:hi].astype(np.float64)
        t0 = theta[lo:hi].astype(np.float64)
        A = (wk[None, None, :] * np.sqrt(np.pi)
             / (2.0 * LPER * s[:, :, None] ** 3)
             * np.exp(-wk[None, None, :] ** 2 / (4.0 * s[:, :, None] ** 2)))
        ph = wk[None, None, :] * t0[:, :, None]
        SC[lo:hi] = (a[:, :, None] * 2.0 * A * np.cos(ph)).sum(1)
        CC[lo:hi] = -(a[:, :, None] * 2.0 * A * np.sin(ph)).sum(1)
    return SC, CC


def _corr_coeffs(amp2, sg, theta, slot_mask):
    """bf16 hi/lo split rows for one narrow slot.

    x rows (8): s1*t1, s1*t2, s1*t3, s2*t1, s2*t2, c1, c2, c3
    adth rows (5): a1*t1, a1*t2, a2*t1, cn1, cn2
    amp2 = -H*a*sqrt(pi)/2 (DErf fold). Zeroed where slot_mask False."""
    sgm = np.where(slot_mask, sg, 1.0)
    amp = np.where(slot_mask, amp2, 0.0)
    th0 = np.where(slot_mask, theta, 0.0)
    s1 = _bf16(sgm).astype(np.float64)
    s2 = _bf16(sgm - s1).astype(np.float64)
    sgr = s1 + s2
    cxx = -sgr * th0
    c1 = _bf16(cxx).astype(np.float64)
    c2 = _bf16(cxx - c1).astype(np.float64)
    c3 = _bf16(cxx - c1 - c2).astype(np.float64)
    a1 = _bf16(amp).astype(np.float64)
    a2 = _bf16(amp - a1).astype(np.float64)
    cn = -(a1 + a2) * th0
    cn1 = _bf16(cn).astype(np.float64)
    cn2 = _bf16(cn - cn1).astype(np.float64)
    s1 = np.where(slot_mask, s1, 0.0)
    s2 = np.where(slot_mask, s2, 0.0)
    return [s1, s1, s1, s2, s2, c1, c2, c3], [a1, a1, a2, cn1, cn2]


def _make_in_maps(x, v0, ntiles=NTILES_FULL):
    import ml_dtypes

    th, z0 = _host_tables()
    W, C, F = _tile_split(ntiles)
    NCOR = C + 3 * F
    GB = 8 if ntiles % 8 == 0 else (4 if ntiles % 4 == 0 else 1)
    rows = ntiles * P
    n_used = N_CORES * rows

    # candidate rows: per-core shard blocks (matches test.py's small-mode
    # expectation layout); full size -> all rows in order
    cand = np.concatenate([np.arange(c * B_SHARD, c * B_SHARD + rows)
                           for c in range(N_CORES)])
    x = np.ascontiguousarray(np.asarray(x, dtype=np.float32))[cand]
    v0 = np.ascontiguousarray(np.asarray(v0, dtype=np.float32))[cand]
    a_all = x[:, 0::3].astype(np.float64)
    b_all = x[:, 1::3].astype(np.float64)
    th_all = x[:, 2::3].astype(np.float64)
    with np.errstate(divide="ignore"):
        sg_all = 1.0 / (SQRT2 * b_all)
    sg_all = np.minimum(sg_all, SG_MAX)

    # classify rows; bump threshold if capacities overflow
    s_thr = S_STAR
    cap_c = N_CORES * C * P
    cap_f = N_CORES * F * P
    cap_w = N_CORES * W * P
    while True:
        narrow = sg_all > s_thr
        nn = narrow.sum(1)
        n_f = int((nn > NSLOT).sum())
        n_c = int(((nn >= 1) & (nn <= NSLOT)).sum())
        if n_f <= cap_f and (n_c + max(0, n_f)) <= cap_c + cap_f and \
           (n_used - n_c - n_f) >= cap_w:
            break
        s_thr *= 1.3
        assert s_thr < SG_MAX * 2, "classification cannot converge"

    wideL = np.where(nn == 0)[0]
    corrL = np.where((nn >= 1) & (nn <= NSLOT))[0]
    fL = np.where(nn > NSLOT)[0]

    # deal narrow rows round-robin per core, then draw wide rows from a
    # global queue so every core gets exactly `rows` rows
    perm = np.empty(n_used, np.int64)       # perm[core*rows + slot] = orig row
    core_rows = []
    wq = list(wideL)[::-1]                  # pop() takes from the front
    for c in range(N_CORES):
        cc = list(corrL[c::N_CORES])
        fc = list(fL[c::N_CORES])
        assert len(fc) <= F * P, "overflow tile capacity exceeded"
        f_slots = fc[:]
        while len(f_slots) < F * P:
            f_slots.append(cc.pop() if cc else wq.pop())
        assert len(cc) <= C * P, "corr tile capacity exceeded"
        c_slots = cc[:]
        while len(c_slots) < C * P:
            c_slots.append(wq.pop())
        w_slots = [wq.pop() for _ in range(W * P)]
        # scatter into the interleaved tile layout
        types, _cr = _tile_layout(ntiles)
        order = np.empty(rows, np.int64)
        wi = ci = 0
        for g in range(ntiles):
            if types[g] == 'W':
                order[g * P:(g + 1) * P] = w_slots[wi * P:(wi + 1) * P]
                wi += 1
            elif types[g] == 'C':
                order[g * P:(g + 1) * P] = c_slots[ci * P:(ci + 1) * P]
                ci += 1
            else:
                order[g * P:(g + 1) * P] = f_slots
        core_rows.append(order)
        perm[c * rows:(c + 1) * rows] = order
    assert not wq, f"{len(wq)} wide rows left over"

    # host tables for the device program
    SG = 4
    NTP = 256
    wk = (2.0 * np.pi / LPER) * np.arange(1, KF + 1)
    sinT = np.sin(np.outer(wk, th.astype(np.float64)))
    cosT = np.cos(np.outer(wk, th.astype(np.float64)))
    hz0 = np.float64(H) * z0.astype(np.float64)
    b1 = np.zeros((P, NTP), np.float16)
    b1[0:KF, 0:NT] = sinT.astype(np.float16)
    b1[KF, 0:NT] = (hz0 * 256.0).astype(np.float16)
    b2 = np.zeros((P, NTP), np.float16)
    b2[0:KF, 0:NT] = cosT.astype(np.float16)
    b2[KF, 0] = np.float16(1.0)          # v0*c delta row

    # segmented scan multiplier pattern: 0 at tile start, c in-tile, 1 pads
    cval = np.float32(1.0) - np.float32(H)
    cb4 = np.full((P, SG * NTP), 1.0, np.float32)
    for j in range(SG):
        cb4[:, j * NTP:j * NTP + NT] = cval
        cb4[:, j * NTP] = 0.0

    t1 = _bf16(th.astype(np.float64)).astype(np.float64)
    t2 = _bf16(th - t1).astype(np.float64)
    t3 = _bf16(th - t1 - t2).astype(np.float64)
    ones = np.ones(NT, np.float64)
    xrows = [t1, t2, t3, t1, t2, ones, ones, ones]
    arows = [t1, t2, t1, ones, ones]
    cbas = np.zeros((KC, 4 * NT), np.float64)
    for s in range(NSLOT):
        for r in range(KX):
            cbas[s * KX + r, s * NT:(s + 1) * NT] = xrows[r]
        for r in range(KA):
            cbas[NSLOT * KX + s * KA + r,
                 (NSLOT + s) * NT:(NSLOT + s + 1) * NT] = arows[r]
    cbas = _bf16(cbas)

    in_maps = []
    for c in range(N_CORES):
        ridx = core_rows[c]
        a_t = a_all[ridx]
        sg_t = sg_all[ridx]
        th_t = th_all[ridx]
        nar_t = sg_t > s_thr
        amp = (-np.float64(H)) * a_t
        wide_t = ~nar_t

        SC, CC = _fourier_coeffs(amp, sg_t, th_t, wide_t)
        lhs1 = np.zeros((P, rows), np.float16)
        lhs2 = np.zeros((P, rows), np.float16)
        # slot s = g*128+p ; lhs[k, s] = SC[s, k]
        lhs1[0:KF] = SC.T.astype(np.float16)
        lhs1[KF] = np.float16(1.0 / 256.0)
        lhs2[0:KF] = CC.T.astype(np.float16)
        lhs2[KF] = (np.float64(cval) * v0[ridx]).astype(np.float16)

        # corr stationaries
        amp2 = (-np.float64(H) * SPI2) * a_t
        clhs = np.zeros((KC, NCOR * P), ml_dtypes.bfloat16)
        # narrow wave indices per row, padded
        nar_idx = [np.where(nar_t[r])[0] for r in range(rows)]

        def fill_slice(idx, row_ids, slot_pair):
            """stationary slice idx covers rows row_ids (128), slots
            slot_pair = (s0, s1) wave-position selector per row."""
            m = len(row_ids)
            sgv = np.ones((m, NSLOT))
            ampv = np.zeros((m, NSLOT))
            thv = np.zeros((m, NSLOT))
            msk = np.zeros((m, NSLOT), bool)
            for r, rid in enumerate(row_ids):
                waves = nar_idx[rid][slot_pair[0]:slot_pair[1]]
                for s, wv in enumerate(waves[:NSLOT]):
                    sgv[r, s] = sg_t[rid, wv]
                    ampv[r, s] = amp2[rid, wv]
                    thv[r, s] = th_t[rid, wv]
                    msk[r, s] = True
            for s in range(NSLOT):
                xr, ar = _corr_coeffs(ampv[:, s], sgv[:, s], thv[:, s],
                                      msk[:, s])
                for r in range(KX):
                    clhs[s * KX + r, idx * P:idx * P + m] = \
                        xr[r].astype(ml_dtypes.bfloat16)
                for r in range(KA):
                    clhs[NSLOT * KX + s * KA + r, idx * P:idx * P + m] = \
                        ar[r].astype(ml_dtypes.bfloat16)

        types, corr_rank = _tile_layout(ntiles)
        for g, rank in corr_rank.items():
            fill_slice(rank, list(range(g * P, (g + 1) * P)), (0, NSLOT))
        gF = ntiles - 1
        for p_ in range(3):
            fill_slice(C + p_, list(range(gF * P, (gF + 1) * P)),
                       (p_ * NSLOT, (p_ + 1) * NSLOT))

        in_maps.append({
            "lhs1": lhs1,
            "lhs2": lhs2,
            "b1": b1,
            "b2": b2,
            "clhs": clhs,
            "cbas": cbas,
            "cb4": cb4,
        })
    return in_maps, perm


def kernel_run(x, v0, trace=False, ntiles=NTILES_FULL):
    """Run the bass kernel; returns (out [B,216] f32, BassKernelResults)."""
    from concourse.bass_utils import run_bass_kernel_spmd

    nc = _get_program(ntiles)
    in_maps, perm = _make_in_maps(x, v0, ntiles)
    res = run_bass_kernel_spmd(nc, in_maps, list(range(N_CORES)), trace=trace)
    dev = np.concatenate(
        [res.results[c]["out"].astype(np.float32) for c in range(N_CORES)],
        axis=0)
    out = np.empty_like(dev)
    out[perm] = dev
    return out, res


def kernel(x, v0):
    out, _ = kernel_run(x, v0)
    return out


# revision 28
# speedup vs baseline: 1.0435x; 1.0435x over previous
"""Trainium2 Bass kernel for the McSharry-style ECG Euler integrator (v3).

Problem (hardcoded): B=131072 beats, params x[B,15] = interleaved (a,b,theta)
x 5 gaussian waves, v0[B] initial z; 216 Euler steps; per-row min/max rescale.

The (x,y) orbit is batch-independent -> th(t), z0(t) are 216-entry host
tables, and per row  z_{t+1} = c*z_t + u_t  with
    u_t = hz0_t - H * sum_i a_i * g_{s_i,theta_i}(th_t),
    g_{s,th0}(th) = (th-th0) * exp(-s^2 (th-th0)^2),  s = min(1/(sqrt2*b), 1e3).

v3 insight: g has an ANALYTIC Fourier transform, so on a period-12 domain
    g(th) = sum_k 2*A_k(s) * sin(w_k (th - th0)),  w_k = 2 pi k / 12,
    A_k = w_k sqrt(pi)/(2 L s^3) e^{-w_k^2/(4 s^2)}
truncated at K=127 harmonics: numerically exact for s <= ~13.  The host
folds the whole 5-wave sum into per-row sin/cos coefficients, so on
device u comes from TWO f16 128x128 matmuls against fixed sin/cos
tables -- no per-wave work at all.  Rows where some s > 13 ("narrow"
waves, ~24%) are sorted into correction tiles: their narrow waves are
evaluated directly via a bf16 hi/lo-split PE matmul (x = s*(th-th0),
adth) + one ACT Derivative_Erf + a short f16 DVE chain, added onto the
spectral u.  Rows with 3+ narrow waves (~0.2%) go to one overflow tile
that runs the correction path three times.

Engine split: PE 2 f16 matmuls/tile (+1 bf16 for corr); ACT DErf (corr)
+ per-tile rescale; DVE z-scan, f16 min/max tensor_reduce per 8-tile
group, corr q-chain.  GPSIMD stays idle -- measured: its SBUF traffic
contends with DVE and slows every DVE op ~3x.
Output f16, upcast on host.  Sharding: data-parallel over 8 cores with
a host-side row permutation (narrow rows dealt evenly), inverted after.
"""

import math
import numpy as np

# ---------------------------------------------------------------- constants
B_FULL = 131072
N_CORES = 8
B_SHARD = B_FULL // N_CORES      # 16384
NT = 216                         # time steps
NW = 5                           # gaussian waves
P = 128                          # partitions
NTILES_FULL = B_SHARD // P       # 128 row-tiles per core
KF = 127                         # Fourier harmonics
LPER = 12.0                      # Fourier period
S_STAR = 13.0                    # narrow-wave threshold
KX = 8                           # x coeff rows per corr slot
KA = 5                           # adth coeff rows per corr slot
NSLOT = 2                        # narrow slots per corr tile row
KC = NSLOT * (KX + KA)           # 26 corr stationary rows

H = 1.0 / 216.0
A_Z0 = 0.005
F2 = 0.25
OMEGA = 2.0 * math.pi
X0 = -0.417750770388669
Y0 = -0.9085616622823985
MIN_VAL = -0.01563
MAX_VAL = 0.042557
SQRT2 = math.sqrt(2.0)
SG_MAX = 1.0e3
SPI2 = math.sqrt(math.pi) / 2.0


def _tile_split(ntiles):
    """(W, C, F) tile counts per core."""
    if ntiles >= 16:
        C = max(2, int(math.ceil(ntiles * 0.265)))
        F = 1
    else:
        C = max(1, ntiles // 4)
        F = 1
    W = ntiles - C - F
    assert W >= 1
    return W, C, F


def _tile_layout(ntiles):
    """Per-tile type layout: corr tiles spread evenly, overflow tile last.
    Returns (types list 'W'/'C'/'F', corr_rank dict g->slice index)."""
    W, C, F = _tile_split(ntiles)
    types = ['W'] * ntiles
    types[ntiles - 1] = 'F'
    pos = [int((i + 0.5) * (ntiles - 1) / C) for i in range(C)]
    # resolve collisions while keeping order
    used = {ntiles - 1}
    corr_pos = []
    for p_ in pos:
        while p_ in used:
            p_ += 1
        assert p_ <= ntiles - 2
        used.add(p_)
        corr_pos.append(p_)
    for p_ in corr_pos:
        types[p_] = 'C'
    corr_rank = {g: i for i, g in enumerate(sorted(corr_pos))}
    return types, corr_rank


def _host_tables():
    """Replicate the reference's fp32 (x,y) Euler orbit -> th, z0 tables."""
    h = np.float32(H)
    om = np.float32(OMEGA)
    one = np.float32(1.0)
    x = np.float32(X0)
    y = np.float32(Y0)
    th = np.empty(NT, np.float32)
    for k in range(NT):
        th[k] = np.arctan2(y, x)
        r = np.sqrt(x * x + y * y)
        alpha = one - r
        fx = alpha * x - om * y
        fy = alpha * y + om * x
        x = x + h * fx
        y = y + h * fy
    t = np.arange(NT, dtype=np.float32) / np.float32(216.0)
    z0 = np.float32(A_Z0) * np.sin(np.float32(2.0 * math.pi * F2) * t)
    return th, z0


def _build_program(ntiles=NTILES_FULL):
    import concourse.bacc as bacc
    import concourse.tile as tile
    from concourse import mybir

    f32 = mybir.dt.float32
    f16 = mybir.dt.float16
    bf16 = mybir.dt.bfloat16
    Act = mybir.ActivationFunctionType
    Op = mybir.AluOpType
    X = mybir.AxisListType.X

    rows = ntiles * P
    W, C, F = _tile_split(ntiles)
    NCOR = C + 3 * F                 # corr stationary slices (F = 3 passes)
    GB = 8 if ntiles % 8 == 0 else (4 if ntiles % 4 == 0 else 1)
    assert ntiles % GB == 0

    nc = bacc.Bacc("TRN2", target_bir_lowering=False, debug=False,
                   num_devices=N_CORES)

    SG = 4                           # tiles per batched scan (PSUM group)
    NTP = 256                        # padded per-tile column slot
    assert ntiles % SG == 0 and GB % SG == 0

    lhs1_d = nc.declare_dram_parameter("lhs1", [P, rows], f16, isOutput=False)
    lhs2_d = nc.declare_dram_parameter("lhs2", [P, rows], f16, isOutput=False)
    b1_d = nc.declare_dram_parameter("b1", [P, NTP], f16, isOutput=False)
    b2_d = nc.declare_dram_parameter("b2", [P, NTP], f16, isOutput=False)
    clhs_d = nc.declare_dram_parameter("clhs", [KC, NCOR * P], bf16, isOutput=False)
    cbas_d = nc.declare_dram_parameter("cbas", [KC, 4 * NT], bf16, isOutput=False)
    cb4_d = nc.declare_dram_parameter("cb4", [P, SG * NT], f32, isOutput=False)
    out_d = nc.declare_dram_parameter("out", [rows, NT], f16, isOutput=True)

    with tile.TileContext(nc) as tc:
        with tc.tile_pool(name="consts", bufs=1) as consts, \
             tc.tile_pool(name="work", bufs=6) as work, \
             tc.tile_pool(name="zp", bufs=3) as zp, \
             tc.tile_pool(name="outp", bufs=3) as outp, \
             tc.tile_pool(name="ups", bufs=2, space="PSUM") as upsp, \
             tc.tile_pool(name="cps", bufs=2, space="PSUM") as cpsp:

            NCH = 16 if ntiles % 16 == 0 else 1
            chunk = rows // NCH
            LHS1c = []
            LHS2c = []
            for cc in range(NCH):
                l1 = consts.tile([P, chunk], f16, name=f"lhs1c{cc}")
                nc.sync.dma_start(out=l1,
                                  in_=lhs1_d[:, cc * chunk:(cc + 1) * chunk])
                LHS1c.append(l1)
                l2 = consts.tile([P, chunk], f16, name=f"lhs2c{cc}")
                nc.sync.dma_start(out=l2,
                                  in_=lhs2_d[:, cc * chunk:(cc + 1) * chunk])
                LHS2c.append(l2)
            tpc = ntiles // NCH              # tiles per chunk

            def lhs_slice(lst, g):
                return lst[g // tpc][:, (g % tpc) * P:(g % tpc + 1) * P]

            B1 = consts.tile([P, NTP], f16)
            nc.sync.dma_start(out=B1, in_=b1_d[:, :])
            B2 = consts.tile([P, NTP], f16)
            nc.sync.dma_start(out=B2, in_=b2_d[:, :])
            CLHS = consts.tile([KC, NCOR * P], bf16)
            nc.sync.dma_start(out=CLHS, in_=clhs_d[:, :])
            CBAS = consts.tile([KC, 4 * NT], bf16)
            nc.sync.dma_start(out=CBAS, in_=cbas_d[:, :])
            CB4 = consts.tile([P, SG * NT], f32)
            nc.sync.dma_start(out=CB4, in_=cb4_d[:, :])
            MINV = consts.tile([P, GB], f32)
            nc.vector.memset(MINV, MIN_VAL)

            RING = 4
            e2r = [consts.tile([P, 2 * NT], f16, name=f"e2r{k}")
                   for k in range(RING)]
            q2r = [consts.tile([P, 2 * NT], f16, name=f"q2r{k}")
                   for k in range(RING)]

            def corr_pass(idx, g, ups_slot):
                """Add 2 narrow-wave slots (stationary slice idx) into the
                PSUM u slot [P,NT] of tile g, in place."""
                CPS = cpsp.tile([P, 1024], f32, tag="cps")
                cl = CLHS[:, idx * P:(idx + 1) * P]
                nc.tensor.matmul(CPS[:, 0:432], cl, CBAS[:, 0:432],
                                 start=True, stop=True)
                nc.tensor.matmul(CPS[:, 512:944], cl, CBAS[:, 432:864],
                                 start=True, stop=True)
                e2 = e2r[g % RING]
                nc.scalar.activation(e2, CPS[:, 0:432], Act.Derivative_Erf)
                q2 = q2r[g % RING]
                nc.vector.tensor_mul(q2, CPS[:, 512:944], e2)
                qs = work.tile([P, NT], f16, tag="qs")
                nc.vector.tensor_add(qs, q2[:, 0:NT], q2[:, NT:2 * NT])
                nc.vector.tensor_add(ups_slot, ups_slot, qs)

            types, corr_rank = _tile_layout(ntiles)
            n_used_corr = 0
            for gb in range(ntiles // GB):
                ZR = zp.tile([P, GB, NT], f32, tag="zr")
                for sgi in range(GB // SG):
                    UPSG = upsp.tile([P, SG * NT], f32, tag="ups")
                    for j in range(SG):
                        g = gb * GB + sgi * SG + j
                        l1 = lhs_slice(LHS1c, g)
                        l2 = lhs_slice(LHS2c, g)
                        base = j * NT
                        # split ranges so each matmul stays inside one
                        # 2KB PSUM bank (bank boundary at column 512)
                        lob, hib = base, base + NT
                        cuts = [base] + [b for b in (512,)
                                         if lob < b < hib] + [hib]
                        for ci in range(len(cuts) - 1):
                            lo, hi = cuts[ci], cuts[ci + 1]
                            nc.tensor.matmul(UPSG[:, lo:hi], l1,
                                             B1[:, lo - base:hi - base],
                                             start=True, stop=False)
                            nc.tensor.matmul(UPSG[:, lo:hi], l2,
                                             B2[:, lo - base:hi - base],
                                             start=False, stop=True)
                        ups_u = UPSG[:, base:base + NT]
                        if types[g] == 'C':
                            corr_pass(corr_rank[g], g, ups_u)
                            n_used_corr += 1
                        elif types[g] == 'F':
                            for p_ in range(3):
                                corr_pass(C + p_, g, ups_u)
                                n_used_corr += 1
                    # one segmented scan for SG tiles (v0 injected via the
                    # delta row; c-pattern: 0 at t=0, c in-tile, 1 in pads)
                    nc.vector.tensor_tensor_scan(
                        ZR[:, sgi * SG:(sgi + 1) * SG, :].rearrange(
                            "p j t -> p (j t)"),
                        CB4, UPSG, 0.0, Op.mult, Op.add)

                zmin = work.tile([P, GB], f32, tag="zmin")
                nc.vector.tensor_reduce(zmin, ZR, axis=X, op=Op.min)
                zmax = work.tile([P, GB], f32, tag="zmax")
                nc.vector.tensor_reduce(zmax, ZR, axis=X, op=Op.max)
                d4 = work.tile([P, GB], f32, tag="d4")
                nc.vector.tensor_sub(d4, zmax, zmin)
                r4 = work.tile([P, GB], f32, tag="r4")
                nc.vector.reciprocal(r4, d4)
                s4 = work.tile([P, GB], f32, tag="s4")
                nc.vector.tensor_scalar_mul(s4, r4, MAX_VAL)
                t4 = work.tile([P, GB], f32, tag="t4")
                nc.vector.tensor_mul(t4, zmin, s4)
                bo4 = work.tile([P, GB], f32, tag="bo4")
                nc.vector.tensor_sub(bo4, MINV, t4)
                O8 = outp.tile([P, GB, NT], f16, tag="o8")
                for j in range(GB):
                    nc.scalar.activation(O8[:, j, :], ZR[:, j, :],
                                         Act.Identity,
                                         bias=bo4[:, j:j + 1],
                                         scale=s4[:, j:j + 1])
                nc.sync.dma_start(
                    out=out_d[gb * GB * P:(gb + 1) * GB * P, :].rearrange(
                        "(j p) t -> p j t", p=P),
                    in_=O8)
            assert n_used_corr == NCOR

    nc.compile()
    return nc


_PROG_CACHE = {}


def _get_program(ntiles=NTILES_FULL):
    if ntiles not in _PROG_CACHE:
        _PROG_CACHE[ntiles] = _build_program(ntiles)
    return _PROG_CACHE[ntiles]


def _bf16(x):
    import ml_dtypes
    return np.asarray(x).astype(ml_dtypes.bfloat16)


def _fourier_coeffs(amp, sg, theta, wide):
    """SC, CC [nrows, KF] f32: per-row spectral coefficients of
    sum_i amp_i * (th-theta_i) exp(-sg_i^2 (th-theta_i)^2) over wide waves."""
    n = amp.shape[0]
    SC = np.zeros((n, KF), np.float32)
    CC = np.zeros((n, KF), np.float32)
    wk = (2.0 * np.pi / LPER) * np.arange(1, KF + 1)
    CH = 8192
    for lo in range(0, n, CH):
        hi = min(lo + CH, n)
        a = (amp[lo:hi] * wide[lo:hi]).astype(np.float64)      # [m,5]
        s = sg[lo:hi].astype(np.float64)
        t0 = theta[lo:hi].astype(np.float64)
        A = (wk[None, None, :] * np.sqrt(np.pi)
             / (2.0 * LPER * s[:, :, None] ** 3)
             * np.exp(-wk[None, None, :] ** 2 / (4.0 * s[:, :, None] ** 2)))
        ph = wk[None, None, :] * t0[:, :, None]
        SC[lo:hi] = (a[:, :, None] * 2.0 * A * np.cos(ph)).sum(1)
        CC[lo:hi] = -(a[:, :, None] * 2.0 * A * np.sin(ph)).sum(1)
    return SC, CC


def _corr_coeffs(amp2, sg, theta, slot_mask):
    """bf16 hi/lo split rows for one narrow slot.

    x rows (8): s1*t1, s1*t2, s1*t3, s2*t1, s2*t2, c1, c2, c3
    adth rows (5): a1*t1, a1*t2, a2*t1, cn1, cn2
    amp2 = -H*a*sqrt(pi)/2 (DErf fold). Zeroed where slot_mask False."""
    sgm = np.where(slot_mask, sg, 1.0)
    amp = np.where(slot_mask, amp2, 0.0)
    th0 = np.where(slot_mask, theta, 0.0)
    s1 = _bf16(sgm).astype(np.float64)
    s2 = _bf16(sgm - s1).astype(np.float64)
    sgr = s1 + s2
    cxx = -sgr * th0
    c1 = _bf16(cxx).astype(np.float64)
    c2 = _bf16(cxx - c1).astype(np.float64)
    c3 = _bf16(cxx - c1 - c2).astype(np.float64)
    a1 = _bf16(amp).astype(np.float64)
    a2 = _bf16(amp - a1).astype(np.float64)
    cn = -(a1 + a2) * th0
    cn1 = _bf16(cn).astype(np.float64)
    cn2 = _bf16(cn - cn1).astype(np.float64)
    s1 = np.where(slot_mask, s1, 0.0)
    s2 = np.where(slot_mask, s2, 0.0)
    return [s1, s1, s1, s2, s2, c1, c2, c3], [a1, a1, a2, cn1, cn2]


def _make_in_maps(x, v0, ntiles=NTILES_FULL):
    import ml_dtypes

    th, z0 = _host_tables()
    W, C, F = _tile_split(ntiles)
    NCOR = C + 3 * F
    GB = 8 if ntiles % 8 == 0 else (4 if ntiles % 4 == 0 else 1)
    rows = ntiles * P
    n_used = N_CORES * rows

    # candidate rows: per-core shard blocks (matches test.py's small-mode
    # expectation layout); full size -> all rows in order
    cand = np.concatenate([np.arange(c * B_SHARD, c * B_SHARD + rows)
                           for c in range(N_CORES)])
    x = np.ascontiguousarray(np.asarray(x, dtype=np.float32))[cand]
    v0 = np.ascontiguousarray(np.asarray(v0, dtype=np.float32))[cand]
    a_all = x[:, 0::3].astype(np.float64)
    b_all = x[:, 1::3].astype(np.float64)
    th_all = x[:, 2::3].astype(np.float64)
    with np.errstate(divide="ignore"):
        sg_all = 1.0 / (SQRT2 * b_all)
    sg_all = np.minimum(sg_all, SG_MAX)

    # classify rows; bump threshold if capacities overflow
    s_thr = S_STAR
    cap_c = N_CORES * C * P
    cap_f = N_CORES * F * P
    cap_w = N_CORES * W * P
    while True:
        narrow = sg_all > s_thr
        nn = narrow.sum(1)
        n_f = int((nn > NSLOT).sum())
        n_c = int(((nn >= 1) & (nn <= NSLOT)).sum())
        if n_f <= cap_f and (n_c + max(0, n_f)) <= cap_c + cap_f and \
           (n_used - n_c - n_f) >= cap_w:
            break
        s_thr *= 1.3
        assert s_thr < SG_MAX * 2, "classification cannot converge"

    wideL = np.where(nn == 0)[0]
    corrL = np.where((nn >= 1) & (nn <= NSLOT))[0]
    fL = np.where(nn > NSLOT)[0]

    # deal narrow rows round-robin per core, then draw wide rows from a
    # global queue so every core gets exactly `rows` rows
    perm = np.empty(n_used, np.int64)       # perm[core*rows + slot] = orig row
    core_rows = []
    wq = list(wideL)[::-1]                  # pop() takes from the front
    for c in range(N_CORES):
        cc = list(corrL[c::N_CORES])
        fc = list(fL[c::N_CORES])
        assert len(fc) <= F * P, "overflow tile capacity exceeded"
        f_slots = fc[:]
        while len(f_slots) < F * P:
            f_slots.append(cc.pop() if cc else wq.pop())
        assert len(cc) <= C * P, "corr tile capacity exceeded"
        c_slots = cc[:]
        while len(c_slots) < C * P:
            c_slots.append(wq.pop())
        w_slots = [wq.pop() for _ in range(W * P)]
        # scatter into the interleaved tile layout
        types, _cr = _tile_layout(ntiles)
        order = np.empty(rows, np.int64)
        wi = ci = 0
        for g in range(ntiles):
            if types[g] == 'W':
                order[g * P:(g + 1) * P] = w_slots[wi * P:(wi + 1) * P]
                wi += 1
            elif types[g] == 'C':
                order[g * P:(g + 1) * P] = c_slots[ci * P:(ci + 1) * P]
                ci += 1
            else:
                order[g * P:(g + 1) * P] = f_slots
        core_rows.append(order)
        perm[c * rows:(c + 1) * rows] = order
    assert not wq, f"{len(wq)} wide rows left over"

    # host tables for the device program
    SG = 4
    NTP = 256
    wk = (2.0 * np.pi / LPER) * np.arange(1, KF + 1)
    sinT = np.sin(np.outer(wk, th.astype(np.float64)))
    cosT = np.cos(np.outer(wk, th.astype(np.float64)))
    hz0 = np.float64(H) * z0.astype(np.float64)
    b1 = np.zeros((P, NTP), np.float16)
    b1[0:KF, 0:NT] = sinT.astype(np.float16)
    b1[KF, 0:NT] = (hz0 * 256.0).astype(np.float16)
    b2 = np.zeros((P, NTP), np.float16)
    b2[0:KF, 0:NT] = cosT.astype(np.float16)
    b2[KF, 0] = np.float16(1.0)          # v0*c delta row

    # segmented scan multiplier pattern: 0 at each tile start, else c
    cval = np.float32(1.0) - np.float32(H)
    cb4 = np.full((P, SG * NT), cval, np.float32)
    for j in range(SG):
        cb4[:, j * NT] = 0.0

    t1 = _bf16(th.astype(np.float64)).astype(np.float64)
    t2 = _bf16(th - t1).astype(np.float64)
    t3 = _bf16(th - t1 - t2).astype(np.float64)
    ones = np.ones(NT, np.float64)
    xrows = [t1, t2, t3, t1, t2, ones, ones, ones]
    arows = [t1, t2, t1, ones, ones]
    cbas = np.zeros((KC, 4 * NT), np.float64)
    for s in range(NSLOT):
        for r in range(KX):
            cbas[s * KX + r, s * NT:(s + 1) * NT] = xrows[r]
        for r in range(KA):
            cbas[NSLOT * KX + s * KA + r,
                 (NSLOT + s) * NT:(NSLOT + s + 1) * NT] = arows[r]
    cbas = _bf16(cbas)

    in_maps = []
    for c in range(N_CORES):
        ridx = core_rows[c]
        a_t = a_all[ridx]
        sg_t = sg_all[ridx]
        th_t = th_all[ridx]
        nar_t = sg_t > s_thr
        amp = (-np.float64(H)) * a_t
        wide_t = ~nar_t

        SC, CC = _fourier_coeffs(amp, sg_t, th_t, wide_t)
        lhs1 = np.zeros((P, rows), np.float16)
        lhs2 = np.zeros((P, rows), np.float16)
        # slot s = g*128+p ; lhs[k, s] = SC[s, k]
        lhs1[0:KF] = SC.T.astype(np.float16)
        lhs1[KF] = np.float16(1.0 / 256.0)
        lhs2[0:KF] = CC.T.astype(np.float16)
        lhs2[KF] = (np.float64(cval) * v0[ridx]).astype(np.float16)

        # corr stationaries
        amp2 = (-np.float64(H) * SPI2) * a_t
        clhs = np.zeros((KC, NCOR * P), ml_dtypes.bfloat16)
        # narrow wave indices per row, padded
        nar_idx = [np.where(nar_t[r])[0] for r in range(rows)]

        def fill_slice(idx, row_ids, slot_pair):
            """stationary slice idx covers rows row_ids (128), slots
            slot_pair = (s0, s1) wave-position selector per row."""
            m = len(row_ids)
            sgv = np.ones((m, NSLOT))
            ampv = np.zeros((m, NSLOT))
            thv = np.zeros((m, NSLOT))
            msk = np.zeros((m, NSLOT), bool)
            for r, rid in enumerate(row_ids):
                waves = nar_idx[rid][slot_pair[0]:slot_pair[1]]
                for s, wv in enumerate(waves[:NSLOT]):
                    sgv[r, s] = sg_t[rid, wv]
                    ampv[r, s] = amp2[rid, wv]
                    thv[r, s] = th_t[rid, wv]
                    msk[r, s] = True
            for s in range(NSLOT):
                xr, ar = _corr_coeffs(ampv[:, s], sgv[:, s], thv[:, s],
                                      msk[:, s])
                for r in range(KX):
                    clhs[s * KX + r, idx * P:idx * P + m] = \
                        xr[r].astype(ml_dtypes.bfloat16)
                for r in range(KA):
                    clhs[NSLOT * KX + s * KA + r, idx * P:idx * P + m] = \
                        ar[r].astype(ml_dtypes.bfloat16)

        types, corr_rank = _tile_layout(ntiles)
        for g, rank in corr_rank.items():
            fill_slice(rank, list(range(g * P, (g + 1) * P)), (0, NSLOT))
        gF = ntiles - 1
        for p_ in range(3):
            fill_slice(C + p_, list(range(gF * P, (gF + 1) * P)),
                       (p_ * NSLOT, (p_ + 1) * NSLOT))

        in_maps.append({
            "lhs1": lhs1,
            "lhs2": lhs2,
            "b1": b1,
            "b2": b2,
            "clhs": clhs,
            "cbas": cbas,
            "cb4": cb4,
        })
    return in_maps, perm


def kernel_run(x, v0, trace=False, ntiles=NTILES_FULL):
    """Run the bass kernel; returns (out [B,216] f32, BassKernelResults)."""
    from concourse.bass_utils import run_bass_kernel_spmd

    nc = _get_program(ntiles)
    in_maps, perm = _make_in_maps(x, v0, ntiles)
    res = run_bass_kernel_spmd(nc, in_maps, list(range(N_CORES)), trace=trace)
    dev = np.concatenate(
        [res.results[c]["out"].astype(np.float32) for c in range(N_CORES)],
        axis=0)
    out = np.empty_like(dev)
    out[perm] = dev
    return out, res


def kernel(x, v0):
    out, _ = kernel_run(x, v0)
    return out


# revision 30
# speedup vs baseline: 1.1606x; 1.1122x over previous
"""Trainium2 Bass kernel for the McSharry-style ECG Euler integrator (v3).

Problem (hardcoded): B=131072 beats, params x[B,15] = interleaved (a,b,theta)
x 5 gaussian waves, v0[B] initial z; 216 Euler steps; per-row min/max rescale.

The (x,y) orbit is batch-independent -> th(t), z0(t) are 216-entry host
tables, and per row  z_{t+1} = c*z_t + u_t  with
    u_t = hz0_t - H * sum_i a_i * g_{s_i,theta_i}(th_t),
    g_{s,th0}(th) = (th-th0) * exp(-s^2 (th-th0)^2),  s = min(1/(sqrt2*b), 1e3).

v3 insight: g has an ANALYTIC Fourier transform, so on a period-12 domain
    g(th) = sum_k 2*A_k(s) * sin(w_k (th - th0)),  w_k = 2 pi k / 12,
    A_k = w_k sqrt(pi)/(2 L s^3) e^{-w_k^2/(4 s^2)}
truncated at K=127 harmonics: numerically exact for s <= ~13.  The host
folds the whole 5-wave sum into per-row sin/cos coefficients, so on
device u comes from TWO f16 128x128 matmuls against fixed sin/cos
tables -- no per-wave work at all.  Rows where some s > 13 ("narrow"
waves, ~24%) are sorted into correction tiles: their narrow waves are
evaluated directly via a bf16 hi/lo-split PE matmul (x = s*(th-th0),
adth) + one ACT Derivative_Erf + a short f16 DVE chain, added onto the
spectral u.  Rows with 3+ narrow waves (~0.2%) go to one overflow tile
that runs the correction path three times.

Engine split: PE 2 f16 matmuls/tile (+1 bf16 for corr); ACT DErf (corr)
+ per-tile rescale; DVE z-scan, f16 min/max tensor_reduce per 8-tile
group, corr q-chain.  GPSIMD stays idle -- measured: its SBUF traffic
contends with DVE and slows every DVE op ~3x.
Output f16, upcast on host.  Sharding: data-parallel over 8 cores with
a host-side row permutation (narrow rows dealt evenly), inverted after.
"""

import math
import numpy as np

# ---------------------------------------------------------------- constants
B_FULL = 131072
N_CORES = 8
B_SHARD = B_FULL // N_CORES      # 16384
NT = 216                         # time steps
NW = 5                           # gaussian waves
P = 128                          # partitions
NTILES_FULL = B_SHARD // P       # 128 row-tiles per core
KF = 127                         # Fourier harmonics
LPER = 12.0                      # Fourier period
S_STAR = 13.0                    # narrow-wave threshold
KX = 8                           # x coeff rows per corr slot
KA = 5                           # adth coeff rows per corr slot
NSLOT = 2                        # narrow slots per corr tile row
KC = NSLOT * (KX + KA)           # 26 corr stationary rows

H = 1.0 / 216.0
A_Z0 = 0.005
F2 = 0.25
OMEGA = 2.0 * math.pi
X0 = -0.417750770388669
Y0 = -0.9085616622823985
MIN_VAL = -0.01563
MAX_VAL = 0.042557
SQRT2 = math.sqrt(2.0)
SG_MAX = 1.0e3
SPI2 = math.sqrt(math.pi) / 2.0


def _tile_split(ntiles):
    """(W, C, F) tile counts per core."""
    if ntiles >= 16:
        C = max(2, int(math.ceil(ntiles * 0.265)))
        F = 1
    else:
        C = max(1, ntiles // 4)
        F = 1
    W = ntiles - C - F
    assert W >= 1
    return W, C, F


def _tile_layout(ntiles):
    """Per-tile type layout: corr tiles spread evenly, overflow tile last.
    Returns (types list 'W'/'C'/'F', corr_rank dict g->slice index)."""
    W, C, F = _tile_split(ntiles)
    types = ['W'] * ntiles
    gF = max(0, ntiles - 9)
    types[gF] = 'F'
    pos = [int((i + 0.5) * (ntiles - 1) / C) for i in range(C)]
    # resolve collisions while keeping order
    used = {gF, ntiles - 1}
    corr_pos = []
    for p_ in pos:
        while p_ in used:
            p_ += 1
        assert p_ <= ntiles - 2
        used.add(p_)
        corr_pos.append(p_)
    for p_ in corr_pos:
        types[p_] = 'C'
    corr_rank = {g: i for i, g in enumerate(sorted(corr_pos))}
    return types, corr_rank


def _host_tables():
    """Replicate the reference's fp32 (x,y) Euler orbit -> th, z0 tables."""
    h = np.float32(H)
    om = np.float32(OMEGA)
    one = np.float32(1.0)
    x = np.float32(X0)
    y = np.float32(Y0)
    th = np.empty(NT, np.float32)
    for k in range(NT):
        th[k] = np.arctan2(y, x)
        r = np.sqrt(x * x + y * y)
        alpha = one - r
        fx = alpha * x - om * y
        fy = alpha * y + om * x
        x = x + h * fx
        y = y + h * fy
    t = np.arange(NT, dtype=np.float32) / np.float32(216.0)
    z0 = np.float32(A_Z0) * np.sin(np.float32(2.0 * math.pi * F2) * t)
    return th, z0


def _build_program(ntiles=NTILES_FULL):
    import concourse.bacc as bacc
    import concourse.tile as tile
    from concourse import mybir

    f32 = mybir.dt.float32
    f16 = mybir.dt.float16
    bf16 = mybir.dt.bfloat16
    Act = mybir.ActivationFunctionType
    Op = mybir.AluOpType
    X = mybir.AxisListType.X

    rows = ntiles * P
    W, C, F = _tile_split(ntiles)
    NCOR = C + 3 * F                 # corr stationary slices (F = 3 passes)
    GB = 8 if ntiles % 8 == 0 else (4 if ntiles % 4 == 0 else 1)
    assert ntiles % GB == 0

    nc = bacc.Bacc("TRN2", target_bir_lowering=False, debug=False,
                   num_devices=N_CORES)

    SG = 4                           # tiles per batched scan (PSUM group)
    NTP = 256                        # padded per-tile column slot
    assert ntiles % SG == 0 and GB % SG == 0

    lhs1_d = nc.declare_dram_parameter("lhs1", [P, rows], f16, isOutput=False)
    lhs2_d = nc.declare_dram_parameter("lhs2", [P, rows], f16, isOutput=False)
    b1_d = nc.declare_dram_parameter("b1", [P, NTP], f16, isOutput=False)
    b2_d = nc.declare_dram_parameter("b2", [P, NTP], f16, isOutput=False)
    clhs_d = nc.declare_dram_parameter("clhs", [KC, NCOR * P], bf16, isOutput=False)
    cbas_d = nc.declare_dram_parameter("cbas", [KC, 4 * NT], bf16, isOutput=False)
    cb4_d = nc.declare_dram_parameter("cb4", [P, SG * NT], f32, isOutput=False)
    out_d = nc.declare_dram_parameter("out", [rows, NT], f16, isOutput=True)

    with tile.TileContext(nc) as tc:
        with tc.tile_pool(name="consts", bufs=1) as consts, \
             tc.tile_pool(name="work", bufs=6) as work, \
             tc.tile_pool(name="zp", bufs=3) as zp, \
             tc.tile_pool(name="outp", bufs=3) as outp, \
             tc.tile_pool(name="ups", bufs=2, space="PSUM") as upsp, \
             tc.tile_pool(name="cps", bufs=2, space="PSUM") as cpsp:

            # small tables first so early tiles aren't stuck behind the
            # 8MB of spectral coefficients on the DMA queues
            B1 = consts.tile([P, NTP], f16)
            nc.sync.dma_start(out=B1, in_=b1_d[:, :])
            B2 = consts.tile([P, NTP], f16)
            nc.sync.dma_start(out=B2, in_=b2_d[:, :])
            CLHS = consts.tile([KC, NCOR * P], bf16)
            nc.sync.dma_start(out=CLHS, in_=clhs_d[:, :])
            CBAS = consts.tile([KC, 4 * NT], bf16)
            nc.sync.dma_start(out=CBAS, in_=cbas_d[:, :])
            CB4 = consts.tile([P, SG * NT], f32)
            nc.sync.dma_start(out=CB4, in_=cb4_d[:, :])

            NCH = 16 if ntiles % 16 == 0 else 1
            chunk = rows // NCH
            LHS1c = []
            LHS2c = []
            for cc in range(NCH):
                l1 = consts.tile([P, chunk], f16, name=f"lhs1c{cc}")
                nc.sync.dma_start(out=l1,
                                  in_=lhs1_d[:, cc * chunk:(cc + 1) * chunk])
                LHS1c.append(l1)
                l2 = consts.tile([P, chunk], f16, name=f"lhs2c{cc}")
                nc.sync.dma_start(out=l2,
                                  in_=lhs2_d[:, cc * chunk:(cc + 1) * chunk])
                LHS2c.append(l2)
            tpc = ntiles // NCH              # tiles per chunk

            def lhs_slice(lst, g):
                return lst[g // tpc][:, (g % tpc) * P:(g % tpc + 1) * P]

            MINV = consts.tile([P, GB], f32)
            nc.vector.memset(MINV, MIN_VAL)

            RING = 4
            e2r = [consts.tile([P, 2 * NT], f16, name=f"e2r{k}")
                   for k in range(RING)]
            q2r = [consts.tile([P, 2 * NT], f16, name=f"q2r{k}")
                   for k in range(RING)]

            def corr_pass(idx, g, ups_slot):
                """Add 2 narrow-wave slots (stationary slice idx) into the
                PSUM u slot [P,NT] of tile g, in place."""
                CPS = cpsp.tile([P, 1024], f32, tag="cps")
                cl = CLHS[:, idx * P:(idx + 1) * P]
                nc.tensor.matmul(CPS[:, 0:432], cl, CBAS[:, 0:432],
                                 start=True, stop=True)
                nc.tensor.matmul(CPS[:, 512:944], cl, CBAS[:, 432:864],
                                 start=True, stop=True)
                e2 = e2r[g % RING]
                nc.scalar.activation(e2, CPS[:, 0:432], Act.Derivative_Erf)
                q2 = q2r[g % RING]
                nc.vector.tensor_mul(q2, CPS[:, 512:944], e2)
                qs = work.tile([P, NT], f16, tag="qs")
                nc.vector.tensor_add(qs, q2[:, 0:NT], q2[:, NT:2 * NT])
                nc.vector.tensor_add(ups_slot, ups_slot, qs)

            types, corr_rank = _tile_layout(ntiles)
            n_used_corr = 0
            for gb in range(ntiles // GB):
                ZR = zp.tile([P, GB, NT], f32, tag="zr")
                for sgi in range(GB // SG):
                    UPSG = upsp.tile([P, SG * NT], f32, tag="ups")
                    for j in range(SG):
                        g = gb * GB + sgi * SG + j
                        l1 = lhs_slice(LHS1c, g)
                        l2 = lhs_slice(LHS2c, g)
                        base = j * NT
                        # split ranges so each matmul stays inside one
                        # 2KB PSUM bank (bank boundary at column 512)
                        lob, hib = base, base + NT
                        cuts = [base] + [b for b in (512,)
                                         if lob < b < hib] + [hib]
                        for ci in range(len(cuts) - 1):
                            lo, hi = cuts[ci], cuts[ci + 1]
                            nc.tensor.matmul(UPSG[:, lo:hi], l1,
                                             B1[:, lo - base:hi - base],
                                             start=True, stop=False)
                            nc.tensor.matmul(UPSG[:, lo:hi], l2,
                                             B2[:, lo - base:hi - base],
                                             start=False, stop=True)
                        ups_u = UPSG[:, base:base + NT]
                        if types[g] == 'C':
                            corr_pass(corr_rank[g], g, ups_u)
                            n_used_corr += 1
                        elif types[g] == 'F':
                            for p_ in range(3):
                                corr_pass(C + p_, g, ups_u)
                                n_used_corr += 1
                    # one segmented scan for SG tiles (v0 injected via the
                    # delta row; c-pattern: 0 at t=0, c in-tile, 1 in pads)
                    nc.vector.tensor_tensor_scan(
                        ZR[:, sgi * SG:(sgi + 1) * SG, :].rearrange(
                            "p j t -> p (j t)"),
                        CB4, UPSG, 0.0, Op.mult, Op.add)

                zmin = work.tile([P, GB], f32, tag="zmin")
                nc.vector.tensor_reduce(zmin, ZR, axis=X, op=Op.min)
                zmax = work.tile([P, GB], f32, tag="zmax")
                nc.vector.tensor_reduce(zmax, ZR, axis=X, op=Op.max)
                d4 = work.tile([P, GB], f32, tag="d4")
                nc.vector.tensor_sub(d4, zmax, zmin)
                r4 = work.tile([P, GB], f32, tag="r4")
                nc.vector.reciprocal(r4, d4)
                s4 = work.tile([P, GB], f32, tag="s4")
                nc.vector.tensor_scalar_mul(s4, r4, MAX_VAL)
                t4 = work.tile([P, GB], f32, tag="t4")
                nc.vector.tensor_mul(t4, zmin, s4)
                bo4 = work.tile([P, GB], f32, tag="bo4")
                nc.vector.tensor_sub(bo4, MINV, t4)
                O8 = outp.tile([P, GB, NT], f16, tag="o8")
                for j in range(GB):
                    nc.scalar.activation(O8[:, j, :], ZR[:, j, :],
                                         Act.Identity,
                                         bias=bo4[:, j:j + 1],
                                         scale=s4[:, j:j + 1])
                nc.sync.dma_start(
                    out=out_d[gb * GB * P:(gb + 1) * GB * P, :].rearrange(
                        "(j p) t -> p j t", p=P),
                    in_=O8)
            assert n_used_corr == NCOR

    nc.compile()
    return nc


_PROG_CACHE = {}


def _get_program(ntiles=NTILES_FULL):
    if ntiles not in _PROG_CACHE:
        _PROG_CACHE[ntiles] = _build_program(ntiles)
    return _PROG_CACHE[ntiles]


def _bf16(x):
    import ml_dtypes
    return np.asarray(x).astype(ml_dtypes.bfloat16)


def _fourier_coeffs(amp, sg, theta, wide):
    """SC, CC [nrows, KF] f32: per-row spectral coefficients of
    sum_i amp_i * (th-theta_i) exp(-sg_i^2 (th-theta_i)^2) over wide waves."""
    n = amp.shape[0]
    SC = np.zeros((n, KF), np.float32)
    CC = np.zeros((n, KF), np.float32)
    wk = (2.0 * np.pi / LPER) * np.arange(1, KF + 1)
    CH = 8192
    for lo in range(0, n, CH):
        hi = min(lo + CH, n)
        a = (amp[lo:hi] * wide[lo:hi]).astype(np.float64)      # [m,5]
        s = sg[lo:hi].astype(np.float64)
        t0 = theta[lo:hi].astype(np.float64)
        A = (wk[None, None, :] * np.sqrt(np.pi)
             / (2.0 * LPER * s[:, :, None] ** 3)
             * np.exp(-wk[None, None, :] ** 2 / (4.0 * s[:, :, None] ** 2)))
        ph = wk[None, None, :] * t0[:, :, None]
        SC[lo:hi] = (a[:, :, None] * 2.0 * A * np.cos(ph)).sum(1)
        CC[lo:hi] = -(a[:, :, None] * 2.0 * A * np.sin(ph)).sum(1)
    return SC, CC


def _corr_coeffs(amp2, sg, theta, slot_mask):
    """bf16 hi/lo split rows for one narrow slot.

    x rows (8): s1*t1, s1*t2, s1*t3, s2*t1, s2*t2, c1, c2, c3
    adth rows (5): a1*t1, a1*t2, a2*t1, cn1, cn2
    amp2 = -H*a*sqrt(pi)/2 (DErf fold). Zeroed where slot_mask False."""
    sgm = np.where(slot_mask, sg, 1.0)
    amp = np.where(slot_mask, amp2, 0.0)
    th0 = np.where(slot_mask, theta, 0.0)
    s1 = _bf16(sgm).astype(np.float64)
    s2 = _bf16(sgm - s1).astype(np.float64)
    sgr = s1 + s2
    cxx = -sgr * th0
    c1 = _bf16(cxx).astype(np.float64)
    c2 = _bf16(cxx - c1).astype(np.float64)
    c3 = _bf16(cxx - c1 - c2).astype(np.float64)
    a1 = _bf16(amp).astype(np.float64)
    a2 = _bf16(amp - a1).astype(np.float64)
    cn = -(a1 + a2) * th0
    cn1 = _bf16(cn).astype(np.float64)
    cn2 = _bf16(cn - cn1).astype(np.float64)
    s1 = np.where(slot_mask, s1, 0.0)
    s2 = np.where(slot_mask, s2, 0.0)
    return [s1, s1, s1, s2, s2, c1, c2, c3], [a1, a1, a2, cn1, cn2]


def _make_in_maps(x, v0, ntiles=NTILES_FULL):
    import ml_dtypes

    th, z0 = _host_tables()
    W, C, F = _tile_split(ntiles)
    NCOR = C + 3 * F
    GB = 8 if ntiles % 8 == 0 else (4 if ntiles % 4 == 0 else 1)
    rows = ntiles * P
    n_used = N_CORES * rows

    # candidate rows: per-core shard blocks (matches test.py's small-mode
    # expectation layout); full size -> all rows in order
    cand = np.concatenate([np.arange(c * B_SHARD, c * B_SHARD + rows)
                           for c in range(N_CORES)])
    x = np.ascontiguousarray(np.asarray(x, dtype=np.float32))[cand]
    v0 = np.ascontiguousarray(np.asarray(v0, dtype=np.float32))[cand]
    a_all = x[:, 0::3].astype(np.float64)
    b_all = x[:, 1::3].astype(np.float64)
    th_all = x[:, 2::3].astype(np.float64)
    with np.errstate(divide="ignore"):
        sg_all = 1.0 / (SQRT2 * b_all)
    sg_all = np.minimum(sg_all, SG_MAX)

    # classify rows; bump threshold if capacities overflow
    s_thr = S_STAR
    cap_c = N_CORES * C * P
    cap_f = N_CORES * F * P
    cap_w = N_CORES * W * P
    while True:
        narrow = sg_all > s_thr
        nn = narrow.sum(1)
        n_f = int((nn > NSLOT).sum())
        n_c = int(((nn >= 1) & (nn <= NSLOT)).sum())
        if n_f <= cap_f and (n_c + max(0, n_f)) <= cap_c + cap_f and \
           (n_used - n_c - n_f) >= cap_w:
            break
        s_thr *= 1.3
        assert s_thr < SG_MAX * 2, "classification cannot converge"

    wideL = np.where(nn == 0)[0]
    corrL = np.where((nn >= 1) & (nn <= NSLOT))[0]
    fL = np.where(nn > NSLOT)[0]

    # deal narrow rows round-robin per core, then draw wide rows from a
    # global queue so every core gets exactly `rows` rows
    perm = np.empty(n_used, np.int64)       # perm[core*rows + slot] = orig row
    core_rows = []
    wq = list(wideL)[::-1]                  # pop() takes from the front
    for c in range(N_CORES):
        cc = list(corrL[c::N_CORES])
        fc = list(fL[c::N_CORES])
        assert len(fc) <= F * P, "overflow tile capacity exceeded"
        f_slots = fc[:]
        while len(f_slots) < F * P:
            f_slots.append(cc.pop() if cc else wq.pop())
        assert len(cc) <= C * P, "corr tile capacity exceeded"
        c_slots = cc[:]
        while len(c_slots) < C * P:
            c_slots.append(wq.pop())
        w_slots = [wq.pop() for _ in range(W * P)]
        # scatter into the interleaved tile layout
        types, _cr = _tile_layout(ntiles)
        order = np.empty(rows, np.int64)
        wi = ci = 0
        for g in range(ntiles):
            if types[g] == 'W':
                order[g * P:(g + 1) * P] = w_slots[wi * P:(wi + 1) * P]
                wi += 1
            elif types[g] == 'C':
                order[g * P:(g + 1) * P] = c_slots[ci * P:(ci + 1) * P]
                ci += 1
            else:
                order[g * P:(g + 1) * P] = f_slots
        core_rows.append(order)
        perm[c * rows:(c + 1) * rows] = order
    assert not wq, f"{len(wq)} wide rows left over"

    # host tables for the device program
    SG = 4
    NTP = 256
    wk = (2.0 * np.pi / LPER) * np.arange(1, KF + 1)
    sinT = np.sin(np.outer(wk, th.astype(np.float64)))
    cosT = np.cos(np.outer(wk, th.astype(np.float64)))
    hz0 = np.float64(H) * z0.astype(np.float64)
    b1 = np.zeros((P, NTP), np.float16)
    b1[0:KF, 0:NT] = sinT.astype(np.float16)
    b1[KF, 0:NT] = (hz0 * 256.0).astype(np.float16)
    b2 = np.zeros((P, NTP), np.float16)
    b2[0:KF, 0:NT] = cosT.astype(np.float16)
    b2[KF, 0] = np.float16(1.0)          # v0*c delta row

    # segmented scan multiplier pattern: 0 at each tile start, else c
    cval = np.float32(1.0) - np.float32(H)
    cb4 = np.full((P, SG * NT), cval, np.float32)
    for j in range(SG):
        cb4[:, j * NT] = 0.0

    t1 = _bf16(th.astype(np.float64)).astype(np.float64)
    t2 = _bf16(th - t1).astype(np.float64)
    t3 = _bf16(th - t1 - t2).astype(np.float64)
    ones = np.ones(NT, np.float64)
    xrows = [t1, t2, t3, t1, t2, ones, ones, ones]
    arows = [t1, t2, t1, ones, ones]
    cbas = np.zeros((KC, 4 * NT), np.float64)
    for s in range(NSLOT):
        for r in range(KX):
            cbas[s * KX + r, s * NT:(s + 1) * NT] = xrows[r]
        for r in range(KA):
            cbas[NSLOT * KX + s * KA + r,
                 (NSLOT + s) * NT:(NSLOT + s + 1) * NT] = arows[r]
    cbas = _bf16(cbas)

    in_maps = []
    for c in range(N_CORES):
        ridx = core_rows[c]
        a_t = a_all[ridx]
        sg_t = sg_all[ridx]
        th_t = th_all[ridx]
        nar_t = sg_t > s_thr
        amp = (-np.float64(H)) * a_t
        wide_t = ~nar_t

        SC, CC = _fourier_coeffs(amp, sg_t, th_t, wide_t)
        lhs1 = np.zeros((P, rows), np.float16)
        lhs2 = np.zeros((P, rows), np.float16)
        # slot s = g*128+p ; lhs[k, s] = SC[s, k]
        lhs1[0:KF] = SC.T.astype(np.float16)
        lhs1[KF] = np.float16(1.0 / 256.0)
        lhs2[0:KF] = CC.T.astype(np.float16)
        lhs2[KF] = (np.float64(cval) * v0[ridx]).astype(np.float16)

        # corr stationaries
        amp2 = (-np.float64(H) * SPI2) * a_t
        clhs = np.zeros((KC, NCOR * P), ml_dtypes.bfloat16)
        # narrow wave indices per row, padded
        nar_idx = [np.where(nar_t[r])[0] for r in range(rows)]

        def fill_slice(idx, row_ids, slot_pair):
            """stationary slice idx covers rows row_ids (128), slots
            slot_pair = (s0, s1) wave-position selector per row."""
            m = len(row_ids)
            sgv = np.ones((m, NSLOT))
            ampv = np.zeros((m, NSLOT))
            thv = np.zeros((m, NSLOT))
            msk = np.zeros((m, NSLOT), bool)
            for r, rid in enumerate(row_ids):
                waves = nar_idx[rid][slot_pair[0]:slot_pair[1]]
                for s, wv in enumerate(waves[:NSLOT]):
                    sgv[r, s] = sg_t[rid, wv]
                    ampv[r, s] = amp2[rid, wv]
                    thv[r, s] = th_t[rid, wv]
                    msk[r, s] = True
            for s in range(NSLOT):
                xr, ar = _corr_coeffs(ampv[:, s], sgv[:, s], thv[:, s],
                                      msk[:, s])
                for r in range(KX):
                    clhs[s * KX + r, idx * P:idx * P + m] = \
                        xr[r].astype(ml_dtypes.bfloat16)
                for r in range(KA):
                    clhs[NSLOT * KX + s * KA + r, idx * P:idx * P + m] = \
                        ar[r].astype(ml_dtypes.bfloat16)

        types, corr_rank = _tile_layout(ntiles)
        for g, rank in corr_rank.items():
            fill_slice(rank, list(range(g * P, (g + 1) * P)), (0, NSLOT))
        gF = types.index('F')
        for p_ in range(3):
            fill_slice(C + p_, list(range(gF * P, (gF + 1) * P)),
                       (p_ * NSLOT, (p_ + 1) * NSLOT))

        in_maps.append({
            "lhs1": lhs1,
            "lhs2": lhs2,
            "b1": b1,
            "b2": b2,
            "clhs": clhs,
            "cbas": cbas,
            "cb4": cb4,
        })
    return in_maps, perm


def kernel_run(x, v0, trace=False, ntiles=NTILES_FULL):
    """Run the bass kernel; returns (out [B,216] f32, BassKernelResults)."""
    from concourse.bass_utils import run_bass_kernel_spmd

    nc = _get_program(ntiles)
    in_maps, perm = _make_in_maps(x, v0, ntiles)
    res = run_bass_kernel_spmd(nc, in_maps, list(range(N_CORES)), trace=trace)
    dev = np.concatenate(
        [res.results[c]["out"].astype(np.float32) for c in range(N_CORES)],
        axis=0)
    out = np.empty_like(dev)
    out[perm] = dev
    return out, res


def kernel(x, v0):
    out, _ = kernel_run(x, v0)
    return out


# revision 32
# speedup vs baseline: 1.1970x; 1.0314x over previous
"""Trainium2 Bass kernel for the McSharry-style ECG Euler integrator (v3).

Problem (hardcoded): B=131072 beats, params x[B,15] = interleaved (a,b,theta)
x 5 gaussian waves, v0[B] initial z; 216 Euler steps; per-row min/max rescale.

The (x,y) orbit is batch-independent -> th(t), z0(t) are 216-entry host
tables, and per row  z_{t+1} = c*z_t + u_t  with
    u_t = hz0_t - H * sum_i a_i * g_{s_i,theta_i}(th_t),
    g_{s,th0}(th) = (th-th0) * exp(-s^2 (th-th0)^2),  s = min(1/(sqrt2*b), 1e3).

v3 insight: g has an ANALYTIC Fourier transform, so on a period-12 domain
    g(th) = sum_k 2*A_k(s) * sin(w_k (th - th0)),  w_k = 2 pi k / 12,
    A_k = w_k sqrt(pi)/(2 L s^3) e^{-w_k^2/(4 s^2)}
truncated at K=127 harmonics: numerically exact for s <= ~13.  The host
folds the whole 5-wave sum into per-row sin/cos coefficients, so on
device u comes from TWO f16 128x128 matmuls against fixed sin/cos
tables -- no per-wave work at all.  Rows where some s > 13 ("narrow"
waves, ~24%) are sorted into correction tiles: their narrow waves are
evaluated directly via a bf16 hi/lo-split PE matmul (x = s*(th-th0),
adth) + one ACT Derivative_Erf + a short f16 DVE chain, added onto the
spectral u.  Rows with 3+ narrow waves (~0.2%) go to one overflow tile
that runs the correction path three times.

Engine split: PE 2 f16 matmuls/tile (+1 bf16 for corr); ACT DErf (corr)
+ per-tile rescale; DVE z-scan, f16 min/max tensor_reduce per 8-tile
group, corr q-chain.  GPSIMD stays idle -- measured: its SBUF traffic
contends with DVE and slows every DVE op ~3x.
Output f16, upcast on host.  Sharding: data-parallel over 8 cores with
a host-side row permutation (narrow rows dealt evenly), inverted after.
"""

import math
import numpy as np

# ---------------------------------------------------------------- constants
B_FULL = 131072
N_CORES = 8
B_SHARD = B_FULL // N_CORES      # 16384
NT = 216                         # time steps
NW = 5                           # gaussian waves
P = 128                          # partitions
NTILES_FULL = B_SHARD // P       # 128 row-tiles per core
KF = 127                         # Fourier harmonics
LPER = 12.0                      # Fourier period
S_STAR = 13.0                    # narrow-wave threshold
KX = 8                           # x coeff rows per corr slot
KA = 5                           # adth coeff rows per corr slot
NSLOT = 2                        # narrow slots per corr tile row
KC = NSLOT * (KX + KA)           # 26 corr stationary rows

H = 1.0 / 216.0
A_Z0 = 0.005
F2 = 0.25
OMEGA = 2.0 * math.pi
X0 = -0.417750770388669
Y0 = -0.9085616622823985
MIN_VAL = -0.01563
MAX_VAL = 0.042557
SQRT2 = math.sqrt(2.0)
SG_MAX = 1.0e3
SPI2 = math.sqrt(math.pi) / 2.0


def _tile_split(ntiles):
    """(W, C, F) tile counts per core."""
    if ntiles >= 16:
        C = max(2, int(math.ceil(ntiles * 0.265)))
        F = 1
    else:
        C = max(1, ntiles // 4)
        F = 1
    W = ntiles - C - F
    assert W >= 1
    return W, C, F


def _tile_layout(ntiles):
    """Per-tile type layout: corr tiles spread evenly, overflow tile last.
    Returns (types list 'W'/'C'/'F', corr_rank dict g->slice index)."""
    W, C, F = _tile_split(ntiles)
    types = ['W'] * ntiles
    gF = max(0, ntiles - 9)
    types[gF] = 'F'
    pos = [int((i + 0.5) * (ntiles - 1) / C) for i in range(C)]
    # resolve collisions while keeping order
    used = {gF, ntiles - 1}
    corr_pos = []
    for p_ in pos:
        while p_ in used:
            p_ += 1
        assert p_ <= ntiles - 2
        used.add(p_)
        corr_pos.append(p_)
    for p_ in corr_pos:
        types[p_] = 'C'
    corr_rank = {g: i for i, g in enumerate(sorted(corr_pos))}
    return types, corr_rank


def _host_tables():
    """Replicate the reference's fp32 (x,y) Euler orbit -> th, z0 tables."""
    h = np.float32(H)
    om = np.float32(OMEGA)
    one = np.float32(1.0)
    x = np.float32(X0)
    y = np.float32(Y0)
    th = np.empty(NT, np.float32)
    for k in range(NT):
        th[k] = np.arctan2(y, x)
        r = np.sqrt(x * x + y * y)
        alpha = one - r
        fx = alpha * x - om * y
        fy = alpha * y + om * x
        x = x + h * fx
        y = y + h * fy
    t = np.arange(NT, dtype=np.float32) / np.float32(216.0)
    z0 = np.float32(A_Z0) * np.sin(np.float32(2.0 * math.pi * F2) * t)
    return th, z0


def _build_program(ntiles=NTILES_FULL):
    import concourse.bacc as bacc
    import concourse.tile as tile
    from concourse import mybir

    f32 = mybir.dt.float32
    f16 = mybir.dt.float16
    bf16 = mybir.dt.bfloat16
    Act = mybir.ActivationFunctionType
    Op = mybir.AluOpType
    X = mybir.AxisListType.X

    rows = ntiles * P
    W, C, F = _tile_split(ntiles)
    NCOR = C + 3 * F                 # corr stationary slices (F = 3 passes)
    GB = 8 if ntiles % 8 == 0 else (4 if ntiles % 4 == 0 else 1)
    assert ntiles % GB == 0

    nc = bacc.Bacc("TRN2", target_bir_lowering=False, debug=False,
                   num_devices=N_CORES)

    SG = 4                           # tiles per batched scan (PSUM group)
    NTP = 256                        # padded per-tile column slot
    assert ntiles % SG == 0 and GB % SG == 0

    lhs1_d = nc.declare_dram_parameter("lhs1", [P, rows], f16, isOutput=False)
    lhs2_d = nc.declare_dram_parameter("lhs2", [P, rows], f16, isOutput=False)
    b1_d = nc.declare_dram_parameter("b1", [P, NTP], f16, isOutput=False)
    b2_d = nc.declare_dram_parameter("b2", [P, NTP], f16, isOutput=False)
    clhs_d = nc.declare_dram_parameter("clhs", [KC, NCOR * P], bf16, isOutput=False)
    cbas_d = nc.declare_dram_parameter("cbas", [KC, 4 * NT], bf16, isOutput=False)
    cb4_d = nc.declare_dram_parameter("cb4", [P, SG * NT], f32, isOutput=False)
    id_d = nc.declare_dram_parameter("identh", [P, P], f16, isOutput=False)
    out_d = nc.declare_dram_parameter("out", [rows, NT], f16, isOutput=True)

    with tile.TileContext(nc) as tc:
        with tc.tile_pool(name="consts", bufs=1) as consts, \
             tc.tile_pool(name="work", bufs=6) as work, \
             tc.tile_pool(name="zp", bufs=3) as zp, \
             tc.tile_pool(name="outp", bufs=3) as outp, \
             tc.tile_pool(name="ups", bufs=2, space="PSUM") as upsp, \
             tc.tile_pool(name="cps", bufs=2, space="PSUM") as cpsp:

            # small tables first so early tiles aren't stuck behind the
            # 8MB of spectral coefficients on the DMA queues
            B1 = consts.tile([P, NTP], f16)
            nc.sync.dma_start(out=B1, in_=b1_d[:, :])
            B2 = consts.tile([P, NTP], f16)
            nc.sync.dma_start(out=B2, in_=b2_d[:, :])
            CLHS = consts.tile([KC, NCOR * P], bf16)
            nc.sync.dma_start(out=CLHS, in_=clhs_d[:, :])
            CBAS = consts.tile([KC, 4 * NT], bf16)
            nc.sync.dma_start(out=CBAS, in_=cbas_d[:, :])
            CB4 = consts.tile([P, SG * NT], f32)
            nc.sync.dma_start(out=CB4, in_=cb4_d[:, :])
            IDH = consts.tile([P, P], f16)
            nc.sync.dma_start(out=IDH, in_=id_d[:, :])

            NCH = 16 if ntiles % 16 == 0 else 1
            chunk = rows // NCH
            LHS1c = []
            LHS2c = []
            for cc in range(NCH):
                l1 = consts.tile([P, chunk], f16, name=f"lhs1c{cc}")
                nc.sync.dma_start(out=l1,
                                  in_=lhs1_d[:, cc * chunk:(cc + 1) * chunk])
                LHS1c.append(l1)
                l2 = consts.tile([P, chunk], f16, name=f"lhs2c{cc}")
                nc.sync.dma_start(out=l2,
                                  in_=lhs2_d[:, cc * chunk:(cc + 1) * chunk])
                LHS2c.append(l2)
            tpc = ntiles // NCH              # tiles per chunk

            def lhs_slice(lst, g):
                return lst[g // tpc][:, (g % tpc) * P:(g % tpc + 1) * P]

            MINV = consts.tile([P, GB], f32)
            nc.vector.memset(MINV, MIN_VAL)

            RING = 4
            e2r = [consts.tile([P, 2 * NT], f16, name=f"e2r{k}")
                   for k in range(RING)]
            q2r = [consts.tile([P, 2 * NT], f16, name=f"q2r{k}")
                   for k in range(RING)]

            def corr_q(idx, ring):
                """q [P, 2*NT] f16 for 2 narrow-wave slots (slice idx)."""
                CPS = cpsp.tile([P, 1024], f32, tag="cps")
                cl = CLHS[:, idx * P:(idx + 1) * P]
                nc.tensor.matmul(CPS[:, 0:432], cl, CBAS[:, 0:432],
                                 start=True, stop=True)
                nc.tensor.matmul(CPS[:, 512:944], cl, CBAS[:, 432:864],
                                 start=True, stop=True)
                e2 = e2r[ring % RING]
                nc.scalar.activation(e2, CPS[:, 0:432], Act.Derivative_Erf)
                q2 = q2r[ring % RING]
                nc.vector.tensor_mul(q2, CPS[:, 512:944], e2)
                return q2

            types, corr_rank = _tile_layout(ntiles)
            n_used_corr = 0
            for gb in range(ntiles // GB):
                ZR = zp.tile([P, GB, NT], f32, tag="zr")
                for sgi in range(GB // SG):
                    UPSG = upsp.tile([P, SG * NT], f32, tag="ups")
                    for j in range(SG):
                        g = gb * GB + sgi * SG + j
                        l1 = lhs_slice(LHS1c, g)
                        l2 = lhs_slice(LHS2c, g)
                        base = j * NT
                        # split ranges so each matmul stays inside one
                        # 2KB PSUM bank (bank boundary at column 512)
                        lob, hib = base, base + NT
                        cuts = [base] + [b for b in (512,)
                                         if lob < b < hib] + [hib]
                        is_w = types[g] == 'W'
                        qs = []
                        if types[g] == 'C':
                            qs = [corr_q(corr_rank[g], g)]
                            n_used_corr += 1
                        elif types[g] == 'F':
                            for p_ in range(3):
                                qs.append(corr_q(C + p_, g + p_))
                                n_used_corr += 1
                        for ci in range(len(cuts) - 1):
                            lo, hi = cuts[ci], cuts[ci + 1]
                            nc.tensor.matmul(UPSG[:, lo:hi], l1,
                                             B1[:, lo - base:hi - base],
                                             start=True, stop=False)
                            nc.tensor.matmul(UPSG[:, lo:hi], l2,
                                             B2[:, lo - base:hi - base],
                                             start=False, stop=is_w)
                            # narrow-wave q accumulated straight into PSUM
                            for qi, q2 in enumerate(qs):
                                for half in range(2):
                                    last = (qi == len(qs) - 1 and half == 1)
                                    nc.tensor.matmul(
                                        UPSG[:, lo:hi], IDH,
                                        q2[:, half * NT + lo - base:
                                           half * NT + hi - base],
                                        start=False, stop=last)
                        del base
                    # one segmented scan for SG tiles (v0 injected via the
                    # delta row; c-pattern: 0 at t=0, c in-tile, 1 in pads)
                    nc.vector.tensor_tensor_scan(
                        ZR[:, sgi * SG:(sgi + 1) * SG, :].rearrange(
                            "p j t -> p (j t)"),
                        CB4, UPSG, 0.0, Op.mult, Op.add)

                zmin = work.tile([P, GB], f32, tag="zmin")
                nc.vector.tensor_reduce(zmin, ZR, axis=X, op=Op.min)
                zmax = work.tile([P, GB], f32, tag="zmax")
                nc.vector.tensor_reduce(zmax, ZR, axis=X, op=Op.max)
                d4 = work.tile([P, GB], f32, tag="d4")
                nc.vector.tensor_sub(d4, zmax, zmin)
                r4 = work.tile([P, GB], f32, tag="r4")
                nc.vector.reciprocal(r4, d4)
                s4 = work.tile([P, GB], f32, tag="s4")
                nc.vector.tensor_scalar_mul(s4, r4, MAX_VAL)
                t4 = work.tile([P, GB], f32, tag="t4")
                nc.vector.tensor_mul(t4, zmin, s4)
                bo4 = work.tile([P, GB], f32, tag="bo4")
                nc.vector.tensor_sub(bo4, MINV, t4)
                O8 = outp.tile([P, GB, NT], f16, tag="o8")
                for j in range(GB):
                    nc.scalar.activation(O8[:, j, :], ZR[:, j, :],
                                         Act.Identity,
                                         bias=bo4[:, j:j + 1],
                                         scale=s4[:, j:j + 1])
                    g = gb * GB + j
                    nc.sync.dma_start(out=out_d[g * P:(g + 1) * P, :],
                                      in_=O8[:, j, :])
            assert n_used_corr == NCOR

    nc.compile()
    return nc


_PROG_CACHE = {}


def _get_program(ntiles=NTILES_FULL):
    if ntiles not in _PROG_CACHE:
        _PROG_CACHE[ntiles] = _build_program(ntiles)
    return _PROG_CACHE[ntiles]


def _bf16(x):
    import ml_dtypes
    return np.asarray(x).astype(ml_dtypes.bfloat16)


def _fourier_coeffs(amp, sg, theta, wide):
    """SC, CC [nrows, KF] f32: per-row spectral coefficients of
    sum_i amp_i * (th-theta_i) exp(-sg_i^2 (th-theta_i)^2) over wide waves."""
    n = amp.shape[0]
    SC = np.zeros((n, KF), np.float32)
    CC = np.zeros((n, KF), np.float32)
    wk = (2.0 * np.pi / LPER) * np.arange(1, KF + 1)
    CH = 8192
    for lo in range(0, n, CH):
        hi = min(lo + CH, n)
        a = (amp[lo:hi] * wide[lo:hi]).astype(np.float64)      # [m,5]
        s = sg[lo:hi].astype(np.float64)
        t0 = theta[lo:hi].astype(np.float64)
        A = (wk[None, None, :] * np.sqrt(np.pi)
             / (2.0 * LPER * s[:, :, None] ** 3)
             * np.exp(-wk[None, None, :] ** 2 / (4.0 * s[:, :, None] ** 2)))
        ph = wk[None, None, :] * t0[:, :, None]
        SC[lo:hi] = (a[:, :, None] * 2.0 * A * np.cos(ph)).sum(1)
        CC[lo:hi] = -(a[:, :, None] * 2.0 * A * np.sin(ph)).sum(1)
    return SC, CC


def _corr_coeffs(amp2, sg, theta, slot_mask):
    """bf16 hi/lo split rows for one narrow slot.

    x rows (8): s1*t1, s1*t2, s1*t3, s2*t1, s2*t2, c1, c2, c3
    adth rows (5): a1*t1, a1*t2, a2*t1, cn1, cn2
    amp2 = -H*a*sqrt(pi)/2 (DErf fold). Zeroed where slot_mask False."""
    sgm = np.where(slot_mask, sg, 1.0)
    amp = np.where(slot_mask, amp2, 0.0)
    th0 = np.where(slot_mask, theta, 0.0)
    s1 = _bf16(sgm).astype(np.float64)
    s2 = _bf16(sgm - s1).astype(np.float64)
    sgr = s1 + s2
    cxx = -sgr * th0
    c1 = _bf16(cxx).astype(np.float64)
    c2 = _bf16(cxx - c1).astype(np.float64)
    c3 = _bf16(cxx - c1 - c2).astype(np.float64)
    a1 = _bf16(amp).astype(np.float64)
    a2 = _bf16(amp - a1).astype(np.float64)
    cn = -(a1 + a2) * th0
    cn1 = _bf16(cn).astype(np.float64)
    cn2 = _bf16(cn - cn1).astype(np.float64)
    s1 = np.where(slot_mask, s1, 0.0)
    s2 = np.where(slot_mask, s2, 0.0)
    return [s1, s1, s1, s2, s2, c1, c2, c3], [a1, a1, a2, cn1, cn2]


def _make_in_maps(x, v0, ntiles=NTILES_FULL):
    import ml_dtypes

    th, z0 = _host_tables()
    W, C, F = _tile_split(ntiles)
    NCOR = C + 3 * F
    GB = 8 if ntiles % 8 == 0 else (4 if ntiles % 4 == 0 else 1)
    rows = ntiles * P
    n_used = N_CORES * rows

    # candidate rows: per-core shard blocks (matches test.py's small-mode
    # expectation layout); full size -> all rows in order
    cand = np.concatenate([np.arange(c * B_SHARD, c * B_SHARD + rows)
                           for c in range(N_CORES)])
    x = np.ascontiguousarray(np.asarray(x, dtype=np.float32))[cand]
    v0 = np.ascontiguousarray(np.asarray(v0, dtype=np.float32))[cand]
    a_all = x[:, 0::3].astype(np.float64)
    b_all = x[:, 1::3].astype(np.float64)
    th_all = x[:, 2::3].astype(np.float64)
    with np.errstate(divide="ignore"):
        sg_all = 1.0 / (SQRT2 * b_all)
    sg_all = np.minimum(sg_all, SG_MAX)

    # classify rows; bump threshold if capacities overflow
    s_thr = S_STAR
    cap_c = N_CORES * C * P
    cap_f = N_CORES * F * P
    cap_w = N_CORES * W * P
    while True:
        narrow = sg_all > s_thr
        nn = narrow.sum(1)
        n_f = int((nn > NSLOT).sum())
        n_c = int(((nn >= 1) & (nn <= NSLOT)).sum())
        if n_f <= cap_f and (n_c + max(0, n_f)) <= cap_c + cap_f and \
           (n_used - n_c - n_f) >= cap_w:
            break
        s_thr *= 1.3
        assert s_thr < SG_MAX * 2, "classification cannot converge"

    wideL = np.where(nn == 0)[0]
    corrL = np.where((nn >= 1) & (nn <= NSLOT))[0]
    fL = np.where(nn > NSLOT)[0]

    # deal narrow rows round-robin per core, then draw wide rows from a
    # global queue so every core gets exactly `rows` rows
    perm = np.empty(n_used, np.int64)       # perm[core*rows + slot] = orig row
    core_rows = []
    wq = list(wideL)[::-1]                  # pop() takes from the front
    for c in range(N_CORES):
        cc = list(corrL[c::N_CORES])
        fc = list(fL[c::N_CORES])
        assert len(fc) <= F * P, "overflow tile capacity exceeded"
        f_slots = fc[:]
        while len(f_slots) < F * P:
            f_slots.append(cc.pop() if cc else wq.pop())
        assert len(cc) <= C * P, "corr tile capacity exceeded"
        c_slots = cc[:]
        while len(c_slots) < C * P:
            c_slots.append(wq.pop())
        w_slots = [wq.pop() for _ in range(W * P)]
        # scatter into the interleaved tile layout
        types, _cr = _tile_layout(ntiles)
        order = np.empty(rows, np.int64)
        wi = ci = 0
        for g in range(ntiles):
            if types[g] == 'W':
                order[g * P:(g + 1) * P] = w_slots[wi * P:(wi + 1) * P]
                wi += 1
            elif types[g] == 'C':
                order[g * P:(g + 1) * P] = c_slots[ci * P:(ci + 1) * P]
                ci += 1
            else:
                order[g * P:(g + 1) * P] = f_slots
        core_rows.append(order)
        perm[c * rows:(c + 1) * rows] = order
    assert not wq, f"{len(wq)} wide rows left over"

    # host tables for the device program
    SG = 4
    NTP = 256
    wk = (2.0 * np.pi / LPER) * np.arange(1, KF + 1)
    sinT = np.sin(np.outer(wk, th.astype(np.float64)))
    cosT = np.cos(np.outer(wk, th.astype(np.float64)))
    hz0 = np.float64(H) * z0.astype(np.float64)
    b1 = np.zeros((P, NTP), np.float16)
    b1[0:KF, 0:NT] = sinT.astype(np.float16)
    b1[KF, 0:NT] = (hz0 * 256.0).astype(np.float16)
    b2 = np.zeros((P, NTP), np.float16)
    b2[0:KF, 0:NT] = cosT.astype(np.float16)
    b2[KF, 0] = np.float16(1.0)          # v0*c delta row

    # segmented scan multiplier pattern: 0 at each tile start, else c
    cval = np.float32(1.0) - np.float32(H)
    cb4 = np.full((P, SG * NT), cval, np.float32)
    for j in range(SG):
        cb4[:, j * NT] = 0.0

    t1 = _bf16(th.astype(np.float64)).astype(np.float64)
    t2 = _bf16(th - t1).astype(np.float64)
    t3 = _bf16(th - t1 - t2).astype(np.float64)
    ones = np.ones(NT, np.float64)
    xrows = [t1, t2, t3, t1, t2, ones, ones, ones]
    arows = [t1, t2, t1, ones, ones]
    cbas = np.zeros((KC, 4 * NT), np.float64)
    for s in range(NSLOT):
        for r in range(KX):
            cbas[s * KX + r, s * NT:(s + 1) * NT] = xrows[r]
        for r in range(KA):
            cbas[NSLOT * KX + s * KA + r,
                 (NSLOT + s) * NT:(NSLOT + s + 1) * NT] = arows[r]
    cbas = _bf16(cbas)

    in_maps = []
    for c in range(N_CORES):
        ridx = core_rows[c]
        a_t = a_all[ridx]
        sg_t = sg_all[ridx]
        th_t = th_all[ridx]
        nar_t = sg_t > s_thr
        amp = (-np.float64(H)) * a_t
        wide_t = ~nar_t

        SC, CC = _fourier_coeffs(amp, sg_t, th_t, wide_t)
        lhs1 = np.zeros((P, rows), np.float16)
        lhs2 = np.zeros((P, rows), np.float16)
        # slot s = g*128+p ; lhs[k, s] = SC[s, k]
        lhs1[0:KF] = SC.T.astype(np.float16)
        lhs1[KF] = np.float16(1.0 / 256.0)
        lhs2[0:KF] = CC.T.astype(np.float16)
        lhs2[KF] = (np.float64(cval) * v0[ridx]).astype(np.float16)

        # corr stationaries
        amp2 = (-np.float64(H) * SPI2) * a_t
        clhs = np.zeros((KC, NCOR * P), ml_dtypes.bfloat16)
        # narrow wave indices per row, padded
        nar_idx = [np.where(nar_t[r])[0] for r in range(rows)]

        def fill_slice(idx, row_ids, slot_pair):
            """stationary slice idx covers rows row_ids (128), slots
            slot_pair = (s0, s1) wave-position selector per row."""
            m = len(row_ids)
            sgv = np.ones((m, NSLOT))
            ampv = np.zeros((m, NSLOT))
            thv = np.zeros((m, NSLOT))
            msk = np.zeros((m, NSLOT), bool)
            for r, rid in enumerate(row_ids):
                waves = nar_idx[rid][slot_pair[0]:slot_pair[1]]
                for s, wv in enumerate(waves[:NSLOT]):
                    sgv[r, s] = sg_t[rid, wv]
                    ampv[r, s] = amp2[rid, wv]
                    thv[r, s] = th_t[rid, wv]
                    msk[r, s] = True
            for s in range(NSLOT):
                xr, ar = _corr_coeffs(ampv[:, s], sgv[:, s], thv[:, s],
                                      msk[:, s])
                for r in range(KX):
                    clhs[s * KX + r, idx * P:idx * P + m] = \
                        xr[r].astype(ml_dtypes.bfloat16)
                for r in range(KA):
                    clhs[NSLOT * KX + s * KA + r, idx * P:idx * P + m] = \
                        ar[r].astype(ml_dtypes.bfloat16)

        types, corr_rank = _tile_layout(ntiles)
        for g, rank in corr_rank.items():
            fill_slice(rank, list(range(g * P, (g + 1) * P)), (0, NSLOT))
        gF = types.index('F')
        for p_ in range(3):
            fill_slice(C + p_, list(range(gF * P, (gF + 1) * P)),
                       (p_ * NSLOT, (p_ + 1) * NSLOT))

        in_maps.append({
            "lhs1": lhs1,
            "lhs2": lhs2,
            "b1": b1,
            "b2": b2,
            "clhs": clhs,
            "cbas": cbas,
            "cb4": cb4,
            "identh": np.eye(P, dtype=np.float16),
        })
    return in_maps, perm


def kernel_run(x, v0, trace=False, ntiles=NTILES_FULL):
    """Run the bass kernel; returns (out [B,216] f32, BassKernelResults)."""
    from concourse.bass_utils import run_bass_kernel_spmd

    nc = _get_program(ntiles)
    in_maps, perm = _make_in_maps(x, v0, ntiles)
    res = run_bass_kernel_spmd(nc, in_maps, list(range(N_CORES)), trace=trace)
    dev = np.concatenate(
        [res.results[c]["out"].astype(np.float32) for c in range(N_CORES)],
        axis=0)
    out = np.empty_like(dev)
    out[perm] = dev
    return out, res


def kernel(x, v0):
    out, _ = kernel_run(x, v0)
    return out
